# revision 23
# baseline (speedup 1.0000x reference)
"""Trainium2 Bass kernel for causal multi-head attention with RoPE.

Problem: B=2, S=2048, D=2048, H=16 heads of HD=128.
  q/k/v = x @ w{q,k,v}.T ; RoPE(q,k) ; causal softmax(q k^T/sqrt(HD)) @ v ; @ wo.T

Sharding (8 cores): batch (2) x head-group (4 groups of 4 heads).
Each core: full projections for its 4 heads on its batch, attention, and a
partial output projection (row-shard of wo). Host sums the 4 partials per batch.

Device-side layout tricks (all host-prepared, free at HW time):
  - All streamed tensors are packed partition-major on the host so each
    load is a few large DMAs with fully contiguous per-partition lines
    (128KB chunk sprays measured only ~216 GB/s and convoyed the queues).
  - scores are computed transposed [k,q]: softmax sums via ones-matmul, AV
    yields oT [d,q] whose 128-col slices are exactly the out-proj lhsT.
  - RoPE pairs are pre-permuted into rotate-half layout (even dims in
    partitions 0:64, odd in 64:128) by permuting wq/wk rows on the host.
  - causal masking: upper-triangle k-tiles are skipped entirely; the 4
    diagonal-crossing tile shapes use precomputed 0/1 multiplicative masks.
  - output partials are written bf16 (host accumulates in fp32).

Scheduling notes (hard-won):
  - Q-proj stays in phase B as attention filler. Moving all projections
    into a dense phase A reliably trips the chip into its 2.0GHz P0 power
    state (+20% on every matmul) — measured 399us vs 329us.
  - Rope half-swap DMAs ride the sync queue; gpsimd only does the softmax
    norm partition_broadcasts (sharing one queue convoyed them).
"""

import sys

sys.path.insert(0, "/opt/trn_rl_repo")

from contextlib import ExitStack

import numpy as np
import ml_dtypes

import concourse.bass as bass
import concourse.tile as tile
from concourse import bacc, mybir
from concourse.bass_utils import run_bass_kernel_spmd

B, S, D, H = 2, 2048, 2048, 16
HD, HD2 = 128, 64
NCORES = 8
HPC = 4              # heads per core
DPC = HPC * HD       # 512
GROUPS = H // HPC    # 4 head-groups (x 2 batches = 8 cores)
SCALE = 1.0 / float(np.sqrt(HD))

ST = 512             # q-tile width (free dim of most matmuls)
NST = S // ST        # 4
KT = 128             # k-tile height (partition dim of score tiles)
NKT = S // KT        # 16
NC_CHUNK = D // 128  # 16 contraction chunks for projections

BF16 = mybir.dt.bfloat16
FP16 = mybir.dt.float16
F32 = mybir.dt.float32
NPBF16 = ml_dtypes.bfloat16
NPFP16 = np.float16
LN_INV512 = float(np.log(1.0 / 512.0))  # exp bias so fp16 sums can't overflow

EXP_FN = mybir.ActivationFunctionType.Exp


def build_program(mode: str):
    """mode: 'causal' (skip upper tiles, diag masks), 'dense' (no mask),
    'masked' (multiply every exp tile by a streamed exp(mask) tile)."""
    assert mode in ("causal", "dense", "masked")
    nc = bacc.Bacc(
        "TRN2",
        target_bir_lowering=False,
        debug=False,
        enable_asserts=False,
        num_devices=NCORES,
    )
    xP = nc.dram_tensor("xP", [128, NST, NC_CHUNK, ST], BF16, kind="ExternalInput").ap()
    wqP = nc.dram_tensor("wqP", [128, NC_CHUNK, DPC], BF16, kind="ExternalInput").ap()
    wkP = nc.dram_tensor("wkP", [128, NC_CHUNK, DPC], BF16, kind="ExternalInput").ap()
    wvP = nc.dram_tensor("wvP", [128, NC_CHUNK, DPC], BF16, kind="ExternalInput").ap()
    woP = nc.dram_tensor("woP", [128, HPC, D], BF16, kind="ExternalInput").ap()
    cosT = nc.dram_tensor("cosT", [HD, S], BF16, kind="ExternalInput").ap()
    sinT = nc.dram_tensor("sinT", [HD, S], BF16, kind="ExternalInput").ap()
    maskP = emask = None
    if mode == "causal":
        maskP = nc.dram_tensor("maskP", [KT, 4, ST], FP16, kind="ExternalInput").ap()
    if mode == "masked":
        emask = nc.dram_tensor("emask", [S, S], FP16, kind="ExternalInput").ap()
    out = nc.dram_tensor("out", [S, D], BF16, kind="ExternalOutput").ap()

    with tile.TileContext(nc) as tc, ExitStack() as ctx:
        _body(ctx, tc, mode, xP, wqP, wkP, wvP, woP, cosT, sinT, maskP, emask, out)
    nc.compile()
    return nc


def _body(ctx, tc, mode, xP, wqP, wkP, wvP, woP, cosT, sinT, maskP, emask, out):
    nc = tc.nc
    resid = ctx.enter_context(tc.tile_pool(name="resid", bufs=1))
    xpool = ctx.enter_context(tc.tile_pool(name="xpool", bufs=2))
    psum = ctx.enter_context(tc.tile_pool(name="psum", bufs=1, space="PSUM"))
    tmp = ctx.enter_context(tc.tile_pool(name="tmp", bufs=1))

    # ---- resident weights / constants ----
    # Packed layouts: 4-chunk groups = 4KB contiguous per partition line.
    wq_sb = resid.tile([128, NC_CHUNK, DPC], BF16, name="wq_sb")
    wk_sb = resid.tile([128, NC_CHUNK, DPC], BF16, name="wk_sb")
    wv_sb = resid.tile([128, NC_CHUNK, DPC], BF16, name="wv_sb")
    wo_sb = resid.tile([128, HPC, D], BF16, name="wo_sb")
    cos_sb = resid.tile([128, S], BF16, name="cos_sb")
    sin_sb = resid.tile([128, S], BF16, name="sin_sb")
    mask_sb = None

    x_tiles = {}

    def load_x(st, eng):
        t = xpool.tile([128, NC_CHUNK, ST], BF16, tag="x", bufs=2, name="x_sb")
        for g in range(4):
            eng.dma_start(out=t[:, 4 * g:4 * g + 4, :], in_=xP[:, st, 4 * g:4 * g + 4, :])
        x_tiles[st] = t
        return t

    # Startup: the scalar HWDGE ring lags ~3.5us at kernel start (the ACT
    # table-load DMA occupies it), so the startup-critical wk and x(1)
    # loads ride the sync + vector rings in parallel, interleaved in
    # progressive groups — the first K-proj matmul needs only chunk 0.
    # HAM warmup: the PE clock gate is cold (1.2GHz) until ~3.4us of
    # sustained activity; dummy matmuls during the startup DMA wait
    # flip it to full rate before the first real matmul retires.
    warm_sb = resid.tile([128, 128], BF16, name="warm_sb")
    nc.vector.memset(warm_sb, 0.0)
    for _ in range(28):
        pw = psum.tile([128, 128], F32, tag="pv", bufs=4, name="ps_warm")
        nc.tensor.matmul(pw, warm_sb, warm_sb, start=True, stop=True)

    # Startup rings: the sync HWDGE carries the startup-critical
    # interleaved (wk c, x1 c) pairs for c0-9; the scalar ring (lags
    # ~3.5us behind the ACT table load) takes c10-15 — those aren't
    # consumed until ~12us in, by which point scalar has caught up.
    # gpsimd DMA is a software DGE and measured far too slow for bulk
    # weights (wk-on-gpsimd cost ~24us of early PE stalls).
    x1 = xpool.tile([128, NC_CHUNK, ST], BF16, tag="x", bufs=2, name="x_sb")
    c0 = 0
    for g in (1, 1, 2, 2, 4):
        nc.sync.dma_start(out=wk_sb[:, c0:c0 + g, :], in_=wkP[:, c0:c0 + g, :])
        nc.sync.dma_start(out=x1[:, c0:c0 + g, :], in_=xP[:, 1, c0:c0 + g, :])
        c0 += g
    for g in (2, 4):
        nc.scalar.dma_start(out=wk_sb[:, c0:c0 + g, :], in_=wkP[:, c0:c0 + g, :])
        nc.scalar.dma_start(out=x1[:, c0:c0 + g, :], in_=xP[:, 1, c0:c0 + g, :])
        c0 += g
    x_tiles[1] = x1
    # cos/sin feed the first ropes (~22us) and wv the first V-proj
    # (~24us); mask isn't read until attention (~150us) so it goes last.
    nc.scalar.dma_start(out=cos_sb, in_=cosT)
    nc.scalar.dma_start(out=sin_sb, in_=sinT)
    for g in range(4):
        nc.scalar.dma_start(out=wv_sb[:, 4 * g:4 * g + 4, :], in_=wvP[:, 4 * g:4 * g + 4, :])
    if mode == "causal":
        mask_sb = resid.tile([128, 4, ST], FP16, name="mask_sb")
        nc.scalar.dma_start(out=mask_sb, in_=maskP)

    ones_sb = resid.tile([128, 1], FP16, name="ones_sb")
    nc.vector.memset(ones_sb, 1.0)
    ebias_sb = resid.tile([128, 1], F32, name="ebias_sb")
    nc.vector.memset(ebias_sb, LN_INV512)
    onesf_sb = resid.tile([1, 128], FP16, name="onesf_sb")
    nc.vector.memset(onesf_sb, 1.0)
    # Preload the ACT exp table during the startup DMA wait: the first
    # ACTIVATE(Exp) otherwise pays a ~2.7us lazy table load right at
    # attention start (and the PE gap it causes trips a HAM re-throttle).
    twarm_sb = resid.tile([128, 1], F32, name="twarm_sb")
    nc.scalar.activation(twarm_sb, ebias_sb, func=EXP_FN, scale=1.0)

    # ---- resident activations ----
    qT_sb = resid.tile([128, HPC, S], BF16, name="qT_sb")   # [d, h, q-pos]
    kT_sb = resid.tile([128, HPC, S], BF16, name="kT_sb")   # [d, h, k-pos]
    v_sb = resid.tile([128, NKT, DPC], FP16, name="v_sb")   # [k-pos%128, k-tile, hd]
    oT_sb = resid.tile([128, HPC, S], BF16, name="oT_sb")   # [d, h, q-pos]

    # RoPE (rotate-half layout): dst = t*[c;c] + swap(t)*[-s;s].
    # Engines can't cross partitions, so the half-swap is a DMA.
    # copy_eng: scalar in phase A (ACT idle), vector during attention
    # (ACT is exp-saturated there; DVE has slack).
    def rope(ps, dstT, h, ssl, copy_eng=None):
        stg = tmp.tile([128, ST], BF16, tag="stg", bufs=3, name="stg")
        if copy_eng is nc.vector:
            nc.vector.tensor_copy(stg, ps)
        else:
            nc.scalar.copy(stg, ps)
        swp = tmp.tile([128, ST], BF16, tag="swp", bufs=3, name="swp")
        nc.sync.dma_start(out=swp[0:64, :], in_=stg[64:128, :])
        nc.sync.dma_start(out=swp[64:128, :], in_=stg[0:64, :])
        t1 = tmp.tile([128, ST], BF16, tag="t1", bufs=2, name="t1")
        t2 = tmp.tile([128, ST], BF16, tag="t2", bufs=2, name="t2")
        nc.vector.tensor_mul(t1, stg, cos_sb[:, ssl])
        nc.vector.tensor_mul(t2, swp, sin_sb[:, ssl])
        nc.vector.tensor_add(dstT[:, h, ssl], t1, t2)

    # ========== Phase A: K projection + RoPE, V projection ==========
    # K and V are needed in full before any attention; Q is deferred so its
    # matmuls can hide the exp-heavy attention phase.
    # st=0 goes last so its x tile is still live for qproj(0) in phase B.
    order = (1, 2, 3, 0)
    for idx, st in enumerate(order):
        ssl = slice(st * ST, (st + 1) * ST)
        x_sb = x_tiles[st]
        if idx + 1 < len(order):
            # x(2) on sync (free after startup); x(3)/x(0) on the scalar
            # ring so they don't queue behind the startup interleave
            load_x(order[idx + 1], nc.sync if idx == 0 else nc.scalar)
        psks = []
        for h in range(HPC):
            psks.append(psum.tile([128, ST], F32, tag="pj", bufs=4, name="ps_k"))
        for c in range(NC_CHUNK):
            for h in range(HPC):
                nc.tensor.matmul(
                    psks[h], wk_sb[:, c, h * HD:(h + 1) * HD], x_sb[:, c, :],
                    start=(c == 0), stop=(c == NC_CHUNK - 1),
                )
        for h in range(HPC):
            rope(psks[h], kT_sb, h, ssl)
        for s4 in range(ST // 128):
            pv = psum.tile([128, DPC], F32, tag="pv", bufs=4, name="ps_pv")
            for c in range(NC_CHUNK):
                nc.tensor.matmul(
                    pv, x_sb[:, c, s4 * 128:(s4 + 1) * 128], wv_sb[:, c, :],
                    start=(c == 0), stop=(c == NC_CHUNK - 1),
                )
            nc.vector.tensor_copy(v_sb[:, st * 4 + s4, :], pv)
        if st != 0:
            # only the last-loaded tile (st=0) survives the 2-deep pool
            del x_tiles[st]

    # wq/wo load last on the scalar ring: first needed ~115us/170us in, and
    # emitting them here keeps their completions behind the x prefetches.
    for g in range(4):
        nc.scalar.dma_start(out=wq_sb[:, 4 * g:4 * g + 4, :], in_=wqP[:, 4 * g:4 * g + 4, :])
    nc.scalar.dma_start(out=wo_sb[:, 0:2, :], in_=woP[:, 0:2, :])
    nc.scalar.dma_start(out=wo_sb[:, 2:4, :], in_=woP[:, 2:4, :])

    # ========== Phase B: Q projection interleaved with attention ==========
    # Generators emit filler matmuls (Q-proj / out-proj) that the attention
    # loop interleaves between exp-dependent tiles, keeping PE fed while the
    # scalar engine computes exp.
    def qproj_gen(st):
        ssl = slice(st * ST, (st + 1) * ST)
        if st in x_tiles:
            x_sb = x_tiles.pop(st)
        else:
            x_sb = load_x(st, nc.sync)

        def inner():
            for h in range(HPC):
                psq = psum.tile([128, ST], F32, tag="pj", bufs=4, name="ps_q")
                for c in range(NC_CHUNK):
                    nc.tensor.matmul(
                        psq, wq_sb[:, c, h * HD:(h + 1) * HD], x_sb[:, c, :],
                        start=(c == 0), stop=(c == NC_CHUNK - 1),
                    )
                    yield
                rope(psq, qT_sb, h, ssl, copy_eng=nc.vector)
                yield

        return inner()

    # Softmax norm, split in two stages (stage1: sum matmul + reciprocal;
    # stage2: broadcast + oT multiply). accs is 1-2 partial accumulators
    # (DVE + gpsimd chains); the psm matmul accumulates over them.
    def norm_stage1(h, qt, po, accs):
        psm = psum.tile([1, ST], F32, tag="pv", bufs=4, name="ps_sum")
        for i, a in enumerate(accs):
            nc.tensor.matmul(psm, ones_sb, a,
                             start=(i == 0), stop=(i == len(accs) - 1))
        r_row = tmp.tile([1, ST], F32, tag="r", bufs=2, name="r_row")
        nc.vector.reciprocal_approx_fast(r_row, psm)
        return (h, qt, po, r_row)

    def norm_stage2(st1, use_pe=False):
        h, qt, po, r_row = st1
        qsl = slice(qt * ST, (qt + 1) * ST)
        rb_sb = tmp.tile([128, ST], F32, tag="rb", bufs=2, name="rb_sb")
        if use_pe:
            # final norms sit on the critical tail: PE broadcast is faster
            # than gpsimd's ~3.3us partition_broadcast
            r16 = tmp.tile([1, ST], FP16, tag="r16", bufs=1, name="r16")
            nc.vector.tensor_copy(r16, r_row)
            prb = psum.tile([128, ST], F32, tag="pv", bufs=4, name="ps_rb")
            nc.tensor.matmul(prb, onesf_sb, r16, start=True, stop=True)
            nc.scalar.copy(rb_sb, prb)
        else:
            nc.gpsimd.partition_broadcast(rb_sb, r_row)
        nc.vector.tensor_mul(oT_sb[:, h, qsl], po, rb_sb)

    def emit_norm(h, qt, po, acc, use_pe=False):
        norm_stage2(norm_stage1(h, qt, po, acc), use_pe=use_pe)

    def outproj_gen(qt, wide=False):
        # wide=True (tail only): 4 PSUM tiles from the now-idle pv tag join
        # the rotation so copies never gate the matmul stream.
        def inner():
            for s128 in range(qt * 4, (qt + 1) * 4):
                if wide:
                    pouts = [
                        psum.tile([128, ST], F32, tag="pj", bufs=4, name="ps_out"),
                        psum.tile([128, ST], F32, tag="pj", bufs=4, name="ps_out"),
                        psum.tile([128, ST], F32, tag="pv", bufs=4, name="ps_out"),
                        psum.tile([128, ST], F32, tag="pv", bufs=4, name="ps_out"),
                    ]
                    for h in range(HPC):
                        for j in range(4):
                            nc.tensor.matmul(
                                pouts[j],
                                oT_sb[:, h, s128 * 128:(s128 + 1) * 128],
                                wo_sb[:, h, j * ST:(j + 1) * ST],
                                start=(h == 0), stop=(h == HPC - 1),
                            )
                        yield
                    for j in range(4):
                        o_sb = tmp.tile([128, ST], BF16, tag="osb", bufs=4, name="o_sb")
                        if j % 2 == 0:
                            nc.vector.tensor_copy(o_sb, pouts[j])
                        else:
                            nc.scalar.copy(o_sb, pouts[j])
                        eng = nc.sync if j % 2 == 0 else nc.scalar
                        eng.dma_start(
                            out=out[s128 * 128:(s128 + 1) * 128, j * ST:(j + 1) * ST],
                            in_=o_sb,
                        )
                        yield
                else:
                    for jp in range(2):
                        pouts = []
                        for jj in range(2):
                            pj_ = psum.tile(
                                [128, ST], F32, tag="pj", bufs=4, name="ps_out"
                            )
                            pouts.append(pj_)
                        for h in range(HPC):
                            for jj in range(2):
                                j = 2 * jp + jj
                                nc.tensor.matmul(
                                    pouts[jj],
                                    oT_sb[:, h, s128 * 128:(s128 + 1) * 128],
                                    wo_sb[:, h, j * ST:(j + 1) * ST],
                                    start=(h == 0), stop=(h == HPC - 1),
                                )
                            yield
                        for jj in range(2):
                            j = 2 * jp + jj
                            o_sb = tmp.tile([128, ST], BF16, tag="osb", bufs=4, name="o_sb")
                            # interleaved with attention: keep ACT exp-only
                            nc.vector.tensor_copy(o_sb, pouts[jj])
                            nc.sync.dma_start(
                                out=out[s128 * 128:(s128 + 1) * 128, j * ST:(j + 1) * ST],
                                in_=o_sb,
                            )
                        yield

        return inner()

    def drain(gen):
        for _ in gen:
            pass

    pend = [None]
    pull_acc = [0.0]

    def attn(qt, filler, pulls):
        for h in range(HPC):
            # Flush the previous head's deferred norm FIRST: its psm
            # matmul lands ahead of this head's score stream, so the
            # recip/broadcast/stage2 chain (and the pend-po PSUM bank
            # release) overlaps this whole head instead of trailing it.
            if pend[0] is not None:
                emit_norm(*pend[0])
                pend[0] = None
            nkt = 4 * (qt + 1) if mode == "causal" else NKT
            po = psum.tile([128, ST], F32, tag="pj", bufs=4, name="ps_po")
            # Softmax denominator as 1-2 independent partial chains: DVE
            # is ~90% busy during attention (the serial acc-add chain is
            # its largest load) while gpsimd idles — split even k-tiles
            # to DVE, odd to gpsimd. kt=0/1 are full-width (di<0) for
            # qt>=1 so both chains init by plain copy; qt=0 has only 4
            # (mostly diagonal) tiles and stays on DVE alone.
            split = nkt >= 8
            acc = tmp.tile([128, ST], FP16, tag="acc", bufs=2, name="acc")
            accb = None
            if split:
                accb = tmp.tile([128, ST], FP16, tag="accb", bufs=2, name="accb")

            def emit_av(kt, q0, e_sb, po=po, h=h, nkt=nkt):
                nc.tensor.matmul(
                    po[:, q0:],
                    v_sb[:, kt, h * HD:(h + 1) * HD],
                    e_sb[:, q0:],
                    start=(kt == 0), stop=(kt == nkt - 1),
                )

            prev_av = None
            for kt in range(nkt):
                di = kt - 4 * qt
                # diagonal tiles only have valid queries at columns >= di*KT
                q0 = di * KT if (mode == "causal" and di >= 0) else 0
                qsl = slice(qt * ST + q0, (qt + 1) * ST)
                pss = psum.tile([128, ST], F32, tag="pv", bufs=4, name="ps_s")
                nc.tensor.matmul(
                    pss[:, q0:],
                    kT_sb[:, h, kt * KT:(kt + 1) * KT],
                    qT_sb[:, h, qsl],
                    start=True, stop=True,
                )
                e_sb = tmp.tile([128, ST], FP16, tag="e", bufs=7, name="e_sb")
                nc.scalar.activation(
                    e_sb[:, q0:], pss[:, q0:], func=EXP_FN,
                    scale=SCALE, bias=ebias_sb,
                )
                if mode == "causal":
                    if di >= 0:
                        nc.vector.tensor_mul(
                            e_sb[:, q0:], e_sb[:, q0:], mask_sb[:, di, q0:]
                        )
                elif mode == "masked":
                    m_sb = tmp.tile([128, ST], FP16, tag="m", bufs=4, name="m_sb")
                    nc.sync.dma_start(
                        out=m_sb, in_=emask[kt * KT:(kt + 1) * KT, qsl]
                    )
                    nc.vector.tensor_mul(e_sb, e_sb, m_sb)
                if kt == 0:
                    nc.vector.tensor_copy(acc, e_sb)
                elif split and kt == 1:
                    nc.gpsimd.tensor_copy(accb, e_sb)
                elif split and kt % 2 == 1:
                    nc.gpsimd.tensor_add(accb[:, q0:], accb[:, q0:], e_sb[:, q0:])
                else:
                    nc.vector.tensor_add(acc[:, q0:], acc[:, q0:], e_sb[:, q0:])
                if prev_av is not None:
                    emit_av(*prev_av)
                prev_av = (kt, q0, e_sb)
                pull_acc[0] += pulls
                while pull_acc[0] >= 1.0:
                    pull_acc[0] -= 1.0
                    if next(filler, "end") == "end":
                        pull_acc[0] = 0.0
                        break
            emit_av(*prev_av)
            pend[0] = (h, qt, po, [acc, accb] if split else [acc])

    # pulls is yields-per-score-tile; sized so each generator lasts its
    # whole phase (dry filler = PE waits on the exp chain): yield counts
    # qproj=68, outproj=40 vs tiles 16/32/48/64.
    drain(qproj_gen(0))
    filler = qproj_gen(1)
    attn(0, filler, 4)
    drain(filler)
    filler = qproj_gen(2)
    attn(1, filler, 2)
    drain(filler)
    filler = outproj_gen(0)
    f2 = qproj_gen(3)
    import itertools
    filler = itertools.chain(f2, filler)
    attn(2, filler, 2)
    drain(filler)
    filler = itertools.chain(outproj_gen(1), outproj_gen(2))
    attn(3, filler, 1.25)
    drain(filler)
    emit_norm(*pend[0], use_pe=True)
    drain(outproj_gen(3, wide=True))


# ---------------------------------------------------------------------------
# Host side
# ---------------------------------------------------------------------------

_PROGRAMS: dict = {}


def _get_program(mode: str):
    if mode not in _PROGRAMS:
        _PROGRAMS[mode] = build_program(mode)
    return _PROGRAMS[mode]


_PERM = np.concatenate([np.arange(0, HD, 2), np.arange(1, HD, 2)])  # rotate-half


def _mask4_np() -> np.ndarray:
    m = np.zeros((4, KT, ST), dtype=np.float32)
    p = np.arange(KT)[:, None]
    qf = np.arange(ST)[None, :]
    for di in range(4):
        m[di] = (qf >= di * KT + p).astype(np.float32)
    return m.astype(NPFP16)


def _classify_mask(m: np.ndarray) -> str:
    if not np.any(m):
        return "dense"
    causal = np.triu(np.full((S, S), -1e9, dtype=np.float32), 1)
    if np.array_equal(m, causal):
        return "causal"
    return "masked"


def _pack_w(wT: np.ndarray, groups: int) -> np.ndarray:
    """[groups*128, M] -> [128, groups, M] partition-major contiguous."""
    g, m = groups, wT.shape[1]
    return np.ascontiguousarray(
        wT.reshape(g, 128, m).transpose(1, 0, 2)
    ).astype(NPBF16)


def make_in_maps(x, freqs_cos, freqs_sin, mask, wq, wk, wv, wo, mode):
    """Build the 8 per-core input dicts."""
    cosT = np.ascontiguousarray(np.asarray(freqs_cos, np.float32).T)  # [64, S]
    sinT = np.ascontiguousarray(np.asarray(freqs_sin, np.float32).T)
    cosT2 = np.concatenate([cosT, cosT], 0).astype(NPBF16)            # [128, S]
    # rows 0:64 get -sin (dst_e = qe*c - qo*s), rows 64:128 get +sin
    sinT2 = np.concatenate([-sinT, sinT], 0).astype(NPBF16)
    maskP = None
    if mode == "causal":
        maskP = np.ascontiguousarray(_mask4_np().transpose(1, 0, 2))  # [128,4,512]
    em = None
    if mode == "masked":
        # kernel indexes emask as [k, q]; mask is [q, k]
        em = np.exp(np.asarray(mask, np.float32).reshape(S, S)).T
        em = np.ascontiguousarray(em).astype(NPFP16)

    # permuted rows (within each head) for wq / wk
    perm_rows = (np.arange(H)[:, None] * HD + _PERM[None, :]).reshape(-1)
    wq_p = np.asarray(wq, np.float32)[perm_rows]
    wk_p = np.asarray(wk, np.float32)[perm_rows]
    wv_f = np.asarray(wv, np.float32)
    wo_f = np.asarray(wo, np.float32)
    x_f = np.asarray(x, np.float32)

    # x packed per batch: [128, NST, NC_CHUNK, ST]
    xPs = []
    for b in range(B):
        xPs.append(
            np.ascontiguousarray(
                x_f[b].reshape(NST, ST, NC_CHUNK, 128).transpose(3, 0, 2, 1)
            ).astype(NPBF16)
        )

    in_maps = []
    for core in range(NCORES):
        b, g = divmod(core, GROUPS)
        rs = slice(g * DPC, (g + 1) * DPC)
        im = {
            "xP": xPs[b],
            "wqP": _pack_w(np.ascontiguousarray(wq_p[rs].T), NC_CHUNK),
            "wkP": _pack_w(np.ascontiguousarray(wk_p[rs].T), NC_CHUNK),
            "wvP": _pack_w(np.ascontiguousarray(wv_f[rs].T), NC_CHUNK),
            "woP": _pack_w(np.ascontiguousarray(wo_f[:, rs].T), HPC),
            "cosT": cosT2,
            "sinT": sinT2,
        }
        if mode == "causal":
            im["maskP"] = maskP
        if mode == "masked":
            im["emask"] = em
        in_maps.append(im)
    return in_maps


def assemble(results) -> np.ndarray:
    out = np.zeros((B, S, D), dtype=np.float32)
    for core in range(NCORES):
        b = core // GROUPS
        out[b] += np.asarray(results[core]["out"], dtype=np.float32)
    return out


def kernel(x, freqs_cos, freqs_sin, mask, wq, wk, wv, wo, **run_kwargs):
    mode = _classify_mask(np.asarray(mask, np.float32).reshape(S, S))
    nc = _get_program(mode)
    in_maps = make_in_maps(x, freqs_cos, freqs_sin, mask, wq, wk, wv, wo, mode)
    res = run_bass_kernel_spmd(nc, in_maps, core_ids=list(range(NCORES)), **run_kwargs)
    out = assemble(res.results)
    kernel.last_results = res
    return out



# revision 28
# speedup vs baseline: 1.1724x; 1.1724x over previous
"""Trainium2 Bass kernel for causal multi-head attention with RoPE.

Problem: B=2, S=2048, D=2048, H=16 heads of HD=128.
  q/k/v = x @ w{q,k,v}.T ; RoPE(q,k) ; causal softmax(q k^T/sqrt(HD)) @ v ; @ wo.T

Sharding (8 cores): batch (2) x head-group (4 groups of 4 heads).
Each core: full projections for its 4 heads on its batch, attention, and a
partial output projection (row-shard of wo). Host sums the 4 partials per batch.

Device-side layout tricks (all host-prepared, free at HW time):
  - All streamed tensors are packed partition-major on the host so each
    load is a few large DMAs with fully contiguous per-partition lines
    (128KB chunk sprays measured only ~216 GB/s and convoyed the queues).
  - scores are computed transposed [k,q]: softmax sums via ones-matmul, AV
    yields oT [d,q] whose 128-col slices are exactly the out-proj lhsT.
  - RoPE pairs are pre-permuted into rotate-half layout (even dims in
    partitions 0:64, odd in 64:128) by permuting wq/wk rows on the host.
  - causal masking: upper-triangle k-tiles are skipped entirely; the 4
    diagonal-crossing tile shapes use precomputed 0/1 multiplicative masks.
  - output partials are written bf16 (host accumulates in fp32).

Scheduling notes (hard-won):
  - Q-proj stays in phase B as attention filler. Moving all projections
    into a dense phase A reliably trips the chip into its 2.0GHz P0 power
    state (+20% on every matmul) — measured 399us vs 329us.
  - Rope half-swap DMAs ride the sync queue; gpsimd only does the softmax
    norm partition_broadcasts (sharing one queue convoyed them).
"""

import sys

sys.path.insert(0, "/opt/trn_rl_repo")

from contextlib import ExitStack

import numpy as np
import ml_dtypes

import concourse.bass as bass
import concourse.tile as tile
from concourse import bacc, mybir
from concourse.bass_utils import run_bass_kernel_spmd

B, S, D, H = 2, 2048, 2048, 16
HD, HD2 = 128, 64
NCORES = 8
HPC = 4              # heads per core
DPC = HPC * HD       # 512
GROUPS = H // HPC    # 4 head-groups (x 2 batches = 8 cores)
SCALE = 1.0 / float(np.sqrt(HD))

ST = 512             # q-tile width (free dim of most matmuls)
NST = S // ST        # 4
KT = 128             # k-tile height (partition dim of score tiles)
NKT = S // KT        # 16
NC_CHUNK = D // 128  # 16 contraction chunks for projections

BF16 = mybir.dt.bfloat16
FP16 = mybir.dt.float16
F32 = mybir.dt.float32
NPBF16 = ml_dtypes.bfloat16
NPFP16 = np.float16
LN_INV512 = float(np.log(1.0 / 512.0))  # exp bias so fp16 sums can't overflow

EXP_FN = mybir.ActivationFunctionType.Exp


def build_program(mode: str):
    """mode: 'causal' (skip upper tiles, diag masks), 'dense' (no mask),
    'masked' (multiply every exp tile by a streamed exp(mask) tile)."""
    assert mode in ("causal", "dense", "masked")
    nc = bacc.Bacc(
        "TRN2",
        target_bir_lowering=False,
        debug=False,
        enable_asserts=False,
        num_devices=NCORES,
    )
    xP = nc.dram_tensor("xP", [128, NST, NC_CHUNK, ST], BF16, kind="ExternalInput").ap()
    wqP = nc.dram_tensor("wqP", [128, NC_CHUNK, DPC], BF16, kind="ExternalInput").ap()
    wkP = nc.dram_tensor("wkP", [128, NC_CHUNK, DPC], BF16, kind="ExternalInput").ap()
    wvP = nc.dram_tensor("wvP", [128, NC_CHUNK, DPC], BF16, kind="ExternalInput").ap()
    woP = nc.dram_tensor("woP", [128, HPC, D], BF16, kind="ExternalInput").ap()
    cosT = nc.dram_tensor("cosT", [HD, S], BF16, kind="ExternalInput").ap()
    sinT = nc.dram_tensor("sinT", [HD, S], BF16, kind="ExternalInput").ap()
    maskP = emask = None
    if mode == "causal":
        maskP = nc.dram_tensor("maskP", [KT, 4, ST], FP16, kind="ExternalInput").ap()
    if mode == "masked":
        emask = nc.dram_tensor("emask", [S, S], FP16, kind="ExternalInput").ap()
    out = nc.dram_tensor("out", [S, D], BF16, kind="ExternalOutput").ap()

    with tile.TileContext(nc) as tc, ExitStack() as ctx:
        _body(ctx, tc, mode, xP, wqP, wkP, wvP, woP, cosT, sinT, maskP, emask, out)
    nc.compile()
    return nc


def _body(ctx, tc, mode, xP, wqP, wkP, wvP, woP, cosT, sinT, maskP, emask, out):
    nc = tc.nc
    resid = ctx.enter_context(tc.tile_pool(name="resid", bufs=1))
    xpool = ctx.enter_context(tc.tile_pool(name="xpool", bufs=2))
    psum = ctx.enter_context(tc.tile_pool(name="psum", bufs=1, space="PSUM"))
    tmp = ctx.enter_context(tc.tile_pool(name="tmp", bufs=1))

    # ---- resident weights / constants ----
    # Packed layouts: 4-chunk groups = 4KB contiguous per partition line.
    wq_sb = resid.tile([128, NC_CHUNK, DPC], BF16, name="wq_sb")
    wk_sb = resid.tile([128, NC_CHUNK, DPC], BF16, name="wk_sb")
    wv_sb = resid.tile([128, NC_CHUNK, DPC], BF16, name="wv_sb")
    wo_sb = resid.tile([128, HPC, D], BF16, name="wo_sb")
    cos_sb = resid.tile([128, S], BF16, name="cos_sb")
    sin_sb = resid.tile([128, S], BF16, name="sin_sb")
    mask_sb = None

    x_tiles = {}

    def load_x(st, eng):
        t = xpool.tile([128, NC_CHUNK, ST], BF16, tag="x", bufs=2, name="x_sb")
        for g in range(4):
            eng.dma_start(out=t[:, 4 * g:4 * g + 4, :], in_=xP[:, st, 4 * g:4 * g + 4, :])
        x_tiles[st] = t
        return t

    # Startup: the scalar HWDGE ring lags ~3.5us at kernel start (the ACT
    # table-load DMA occupies it), so the startup-critical wk and x(1)
    # loads ride the sync + vector rings in parallel, interleaved in
    # progressive groups — the first K-proj matmul needs only chunk 0.
    # HAM warmup: the PE clock gate is cold (1.2GHz) until ~3.4us of
    # sustained activity; dummy matmuls during the startup DMA wait
    # flip it to full rate before the first real matmul retires.
    warm_sb = resid.tile([128, 128], BF16, name="warm_sb")
    nc.vector.memset(warm_sb, 0.0)
    for _ in range(28):
        pw = psum.tile([128, 128], F32, tag="pv", bufs=4, name="ps_warm")
        nc.tensor.matmul(pw, warm_sb, warm_sb, start=True, stop=True)

    # Startup rings: the sync HWDGE carries the startup-critical
    # interleaved (wk c, x1 c) pairs for c0-9; the scalar ring (lags
    # ~3.5us behind the ACT table load) takes c10-15 — those aren't
    # consumed until ~12us in, by which point scalar has caught up.
    # gpsimd DMA is a software DGE and measured far too slow for bulk
    # weights (wk-on-gpsimd cost ~24us of early PE stalls).
    x1 = xpool.tile([128, NC_CHUNK, ST], BF16, tag="x", bufs=2, name="x_sb")
    c0 = 0
    for g in (1, 1, 2, 2, 4):
        nc.sync.dma_start(out=wk_sb[:, c0:c0 + g, :], in_=wkP[:, c0:c0 + g, :])
        nc.sync.dma_start(out=x1[:, c0:c0 + g, :], in_=xP[:, 1, c0:c0 + g, :])
        c0 += g
    for g in (2, 4):
        nc.scalar.dma_start(out=wk_sb[:, c0:c0 + g, :], in_=wkP[:, c0:c0 + g, :])
        nc.scalar.dma_start(out=x1[:, c0:c0 + g, :], in_=xP[:, 1, c0:c0 + g, :])
        c0 += g
    x_tiles[1] = x1
    # cos/sin feed the first ropes (~22us) and wv the first V-proj
    # (~24us); mask isn't read until attention (~150us) so it goes last.
    nc.scalar.dma_start(out=cos_sb, in_=cosT)
    nc.scalar.dma_start(out=sin_sb, in_=sinT)
    for g in range(4):
        nc.scalar.dma_start(out=wv_sb[:, 4 * g:4 * g + 4, :], in_=wvP[:, 4 * g:4 * g + 4, :])
    if mode == "causal":
        mask_sb = resid.tile([128, 4, ST], FP16, name="mask_sb")
        nc.scalar.dma_start(out=mask_sb, in_=maskP)

    ones_sb = resid.tile([128, 1], FP16, name="ones_sb")
    nc.vector.memset(ones_sb, 1.0)
    ebias_sb = resid.tile([128, 1], F32, name="ebias_sb")
    nc.vector.memset(ebias_sb, LN_INV512)
    onesf_sb = resid.tile([1, 128], FP16, name="onesf_sb")
    nc.vector.memset(onesf_sb, 1.0)
    # Preload the ACT exp table during the startup DMA wait: the first
    # ACTIVATE(Exp) otherwise pays a ~2.7us lazy table load right at
    # attention start (and the PE gap it causes trips a HAM re-throttle).
    twarm_sb = resid.tile([128, 1], F32, name="twarm_sb")
    nc.scalar.activation(twarm_sb, ebias_sb, func=EXP_FN, scale=1.0)

    # ---- resident activations ----
    qT_sb = resid.tile([128, HPC, S], BF16, name="qT_sb")   # [d, h, q-pos]
    kT_sb = resid.tile([128, HPC, S], BF16, name="kT_sb")   # [d, h, k-pos]
    v_sb = resid.tile([128, NKT, DPC], FP16, name="v_sb")   # [k-pos%128, k-tile, hd]
    oT_sb = resid.tile([128, HPC, S], BF16, name="oT_sb")   # [d, h, q-pos]

    # RoPE (rotate-half layout): dst = t*[c;c] + swap(t)*[-s;s].
    # Engines can't cross partitions, so the half-swap is a DMA.
    # copy_eng: scalar in phase A (ACT idle), vector during attention
    # (ACT is exp-saturated there; DVE has slack).
    def rope(ps, dstT, h, ssl, copy_eng=None):
        stg = tmp.tile([128, ST], BF16, tag="stg", bufs=3, name="stg")
        if copy_eng is nc.vector:
            nc.vector.tensor_copy(stg, ps)
        else:
            nc.scalar.copy(stg, ps)
        swp = tmp.tile([128, ST], BF16, tag="swp", bufs=3, name="swp")
        nc.sync.dma_start(out=swp[0:64, :], in_=stg[64:128, :])
        nc.sync.dma_start(out=swp[64:128, :], in_=stg[0:64, :])
        t1 = tmp.tile([128, ST], BF16, tag="t1", bufs=2, name="t1")
        t2 = tmp.tile([128, ST], BF16, tag="t2", bufs=2, name="t2")
        nc.vector.tensor_mul(t1, stg, cos_sb[:, ssl])
        nc.vector.tensor_mul(t2, swp, sin_sb[:, ssl])
        nc.vector.tensor_add(dstT[:, h, ssl], t1, t2)

    # ========== Phase A: K projection + RoPE, V projection ==========
    # K and V are needed in full before any attention; Q is deferred so its
    # matmuls can hide the exp-heavy attention phase.
    # st=0 goes last so its x tile is still live for qproj(0) in phase B.
    order = (1, 2, 3, 0)
    for idx, st in enumerate(order):
        ssl = slice(st * ST, (st + 1) * ST)
        x_sb = x_tiles[st]
        if idx + 1 < len(order):
            # x(2) on sync (free after startup); x(3)/x(0) on the scalar
            # ring so they don't queue behind the startup interleave
            load_x(order[idx + 1], nc.sync if idx == 0 else nc.scalar)
        psks = []
        for h in range(HPC):
            psks.append(psum.tile([128, ST], F32, tag="pj", bufs=4, name="ps_k"))
        for c in range(NC_CHUNK):
            for h in range(HPC):
                nc.tensor.matmul(
                    psks[h], wk_sb[:, c, h * HD:(h + 1) * HD], x_sb[:, c, :],
                    start=(c == 0), stop=(c == NC_CHUNK - 1),
                )
        for h in range(HPC):
            rope(psks[h], kT_sb, h, ssl)
        for s4 in range(ST // 128):
            pv = psum.tile([128, DPC], F32, tag="pv", bufs=4, name="ps_pv")
            for c in range(NC_CHUNK):
                nc.tensor.matmul(
                    pv, x_sb[:, c, s4 * 128:(s4 + 1) * 128], wv_sb[:, c, :],
                    start=(c == 0), stop=(c == NC_CHUNK - 1),
                )
            nc.vector.tensor_copy(v_sb[:, st * 4 + s4, :], pv)
        if st != 0:
            # only the last-loaded tile (st=0) survives the 2-deep pool
            del x_tiles[st]

    # wq/wo load last on the scalar ring: first needed ~115us/170us in, and
    # emitting them here keeps their completions behind the x prefetches.
    for g in range(4):
        nc.scalar.dma_start(out=wq_sb[:, 4 * g:4 * g + 4, :], in_=wqP[:, 4 * g:4 * g + 4, :])
    nc.scalar.dma_start(out=wo_sb[:, 0:2, :], in_=woP[:, 0:2, :])
    nc.scalar.dma_start(out=wo_sb[:, 2:4, :], in_=woP[:, 2:4, :])

    # ========== Phase B: Q projection interleaved with attention ==========
    # Generators emit filler matmuls (Q-proj / out-proj) that the attention
    # loop interleaves between exp-dependent tiles, keeping PE fed while the
    # scalar engine computes exp.
    def qproj_gen(st):
        ssl = slice(st * ST, (st + 1) * ST)
        if st in x_tiles:
            x_sb = x_tiles.pop(st)
        else:
            x_sb = load_x(st, nc.sync)

        def inner():
            for h in range(HPC):
                psq = psum.tile([128, ST], F32, tag="pj", bufs=4, name="ps_q")
                for c in range(NC_CHUNK):
                    nc.tensor.matmul(
                        psq, wq_sb[:, c, h * HD:(h + 1) * HD], x_sb[:, c, :],
                        start=(c == 0), stop=(c == NC_CHUNK - 1),
                    )
                    yield
                rope(psq, qT_sb, h, ssl, copy_eng=nc.vector)
                yield

        return inner()

    # Softmax norm, split in two stages (stage1: sum matmul + reciprocal;
    # stage2: broadcast + oT multiply). accs is 1-2 partial accumulators
    # (DVE + gpsimd chains); the psm matmul accumulates over them.
    def norm_stage1(h, qt, po, accs):
        psm = psum.tile([1, ST], F32, tag="pv", bufs=4, name="ps_sum")
        for i, a in enumerate(accs):
            nc.tensor.matmul(psm, ones_sb, a,
                             start=(i == 0), stop=(i == len(accs) - 1))
        r_row = tmp.tile([1, ST], F32, tag="r", bufs=2, name="r_row")
        nc.vector.reciprocal_approx_fast(r_row, psm)
        return (h, qt, po, r_row)

    def norm_stage2(st1, use_pe=False):
        h, qt, po, r_row = st1
        qsl = slice(qt * ST, (qt + 1) * ST)
        rb_sb = tmp.tile([128, ST], F32, tag="rb", bufs=2, name="rb_sb")
        if use_pe:
            # final norms sit on the critical tail: PE broadcast is faster
            # than gpsimd's ~3.3us partition_broadcast
            r16 = tmp.tile([1, ST], FP16, tag="r16", bufs=1, name="r16")
            nc.vector.tensor_copy(r16, r_row)
            prb = psum.tile([128, ST], F32, tag="pv", bufs=4, name="ps_rb")
            nc.tensor.matmul(prb, onesf_sb, r16, start=True, stop=True)
            nc.scalar.copy(rb_sb, prb)
        else:
            nc.gpsimd.partition_broadcast(rb_sb, r_row)
        nc.vector.tensor_mul(oT_sb[:, h, qsl], po, rb_sb)

    def emit_norm(h, qt, po, acc, use_pe=False):
        norm_stage2(norm_stage1(h, qt, po, acc), use_pe=use_pe)

    def outproj_gen(qt, wide=False):
        # wide=True (tail only): 4 PSUM tiles from the now-idle pv tag join
        # the rotation so copies never gate the matmul stream.
        def inner():
            for s128 in range(qt * 4, (qt + 1) * 4):
                if wide:
                    pouts = [
                        psum.tile([128, ST], F32, tag="pj", bufs=4, name="ps_out"),
                        psum.tile([128, ST], F32, tag="pj", bufs=4, name="ps_out"),
                        psum.tile([128, ST], F32, tag="pv", bufs=4, name="ps_out"),
                        psum.tile([128, ST], F32, tag="pv", bufs=4, name="ps_out"),
                    ]
                    for h in range(HPC):
                        for j in range(4):
                            nc.tensor.matmul(
                                pouts[j],
                                oT_sb[:, h, s128 * 128:(s128 + 1) * 128],
                                wo_sb[:, h, j * ST:(j + 1) * ST],
                                start=(h == 0), stop=(h == HPC - 1),
                            )
                        yield
                    for j in range(4):
                        o_sb = tmp.tile([128, ST], BF16, tag="osb", bufs=4, name="o_sb")
                        if j % 2 == 0:
                            nc.vector.tensor_copy(o_sb, pouts[j])
                        else:
                            nc.scalar.copy(o_sb, pouts[j])
                        eng = nc.sync if j % 2 == 0 else nc.scalar
                        eng.dma_start(
                            out=out[s128 * 128:(s128 + 1) * 128, j * ST:(j + 1) * ST],
                            in_=o_sb,
                        )
                        yield
                else:
                    for jp in range(2):
                        pouts = []
                        for jj in range(2):
                            pj_ = psum.tile(
                                [128, ST], F32, tag="pj", bufs=4, name="ps_out"
                            )
                            pouts.append(pj_)
                        for h in range(HPC):
                            for jj in range(2):
                                j = 2 * jp + jj
                                nc.tensor.matmul(
                                    pouts[jj],
                                    oT_sb[:, h, s128 * 128:(s128 + 1) * 128],
                                    wo_sb[:, h, j * ST:(j + 1) * ST],
                                    start=(h == 0), stop=(h == HPC - 1),
                                )
                            yield
                        for jj in range(2):
                            j = 2 * jp + jj
                            o_sb = tmp.tile([128, ST], BF16, tag="osb", bufs=4, name="o_sb")
                            if j % 2 == 0:
                                nc.vector.tensor_copy(o_sb, pouts[jj])
                            else:
                                nc.scalar.copy(o_sb, pouts[jj])
                            nc.sync.dma_start(
                                out=out[s128 * 128:(s128 + 1) * 128, j * ST:(j + 1) * ST],
                                in_=o_sb,
                            )
                        yield

        return inner()

    def drain(gen):
        for _ in gen:
            pass

    pend = [None]
    pull_acc = [0.0]

    def attn(qt, filler, pulls):
        for h in range(HPC):
            nkt = 4 * (qt + 1) if mode == "causal" else NKT
            po = psum.tile([128, ST], F32, tag="pj", bufs=4, name="ps_po")
            # (gpsimd acc-split was tried and reverted: gpsimd tensor ops
            # measure 1.1-2us each — the odd-kt chain lagged the ~900ns
            # tile cadence and the norm matmul stalled the in-order PE
            # queue 3-5us per head.)
            acc = tmp.tile([128, ST], FP16, tag="acc", bufs=2, name="acc")

            def emit_av(kt, q0, e_sb, po=po, h=h, nkt=nkt):
                nc.tensor.matmul(
                    po[:, q0:],
                    v_sb[:, kt, h * HD:(h + 1) * HD],
                    e_sb[:, q0:],
                    start=(kt == 0), stop=(kt == nkt - 1),
                )

            prev_av = None
            for kt in range(nkt):
                di = kt - 4 * qt
                # diagonal tiles only have valid queries at columns >= di*KT
                q0 = di * KT if (mode == "causal" and di >= 0) else 0
                qsl = slice(qt * ST + q0, (qt + 1) * ST)
                pss = psum.tile([128, ST], F32, tag="pv", bufs=4, name="ps_s")
                nc.tensor.matmul(
                    pss[:, q0:],
                    kT_sb[:, h, kt * KT:(kt + 1) * KT],
                    qT_sb[:, h, qsl],
                    start=True, stop=True,
                )
                e_sb = tmp.tile([128, ST], FP16, tag="e", bufs=7, name="e_sb")
                nc.scalar.activation(
                    e_sb[:, q0:], pss[:, q0:], func=EXP_FN,
                    scale=SCALE, bias=ebias_sb,
                )
                if mode == "causal":
                    if di >= 0:
                        nc.vector.tensor_mul(
                            e_sb[:, q0:], e_sb[:, q0:], mask_sb[:, di, q0:]
                        )
                elif mode == "masked":
                    m_sb = tmp.tile([128, ST], FP16, tag="m", bufs=4, name="m_sb")
                    nc.sync.dma_start(
                        out=m_sb, in_=emask[kt * KT:(kt + 1) * KT, qsl]
                    )
                    nc.vector.tensor_mul(e_sb, e_sb, m_sb)
                if kt == 0:
                    nc.vector.tensor_copy(acc, e_sb)
                else:
                    nc.vector.tensor_add(acc[:, q0:], acc[:, q0:], e_sb[:, q0:])
                if prev_av is not None:
                    emit_av(*prev_av)
                prev_av = (kt, q0, e_sb)
                if kt == 0 and pend[0] is not None:
                    # Flush the previous head's deferred norm AFTER this
                    # head's first score tile: the psm matmul then never
                    # heads the in-order PE queue while the previous
                    # head's DVE acc chain is still draining.
                    emit_norm(*pend[0])
                    pend[0] = None
                pull_acc[0] += pulls
                while pull_acc[0] >= 1.0:
                    pull_acc[0] -= 1.0
                    if next(filler, "end") == "end":
                        pull_acc[0] = 0.0
                        break
            emit_av(*prev_av)
            pend[0] = (h, qt, po, [acc])

    # pulls is yields-per-score-tile; sized so each generator lasts its
    # whole phase (dry filler = PE waits on the exp chain): yield counts
    # qproj=68, outproj=40 vs tiles 16/32/48/64.
    drain(qproj_gen(0))
    filler = qproj_gen(1)
    attn(0, filler, 4)
    drain(filler)
    filler = qproj_gen(2)
    attn(1, filler, 2)
    drain(filler)
    filler = outproj_gen(0)
    f2 = qproj_gen(3)
    import itertools
    filler = itertools.chain(f2, filler)
    attn(2, filler, 2)
    drain(filler)
    filler = itertools.chain(outproj_gen(1), outproj_gen(2))
    attn(3, filler, 1.25)
    drain(filler)
    emit_norm(*pend[0], use_pe=True)
    drain(outproj_gen(3, wide=True))


# ---------------------------------------------------------------------------
# Host side
# ---------------------------------------------------------------------------

_PROGRAMS: dict = {}


def _get_program(mode: str):
    if mode not in _PROGRAMS:
        _PROGRAMS[mode] = build_program(mode)
    return _PROGRAMS[mode]


_PERM = np.concatenate([np.arange(0, HD, 2), np.arange(1, HD, 2)])  # rotate-half


def _mask4_np() -> np.ndarray:
    m = np.zeros((4, KT, ST), dtype=np.float32)
    p = np.arange(KT)[:, None]
    qf = np.arange(ST)[None, :]
    for di in range(4):
        m[di] = (qf >= di * KT + p).astype(np.float32)
    return m.astype(NPFP16)


def _classify_mask(m: np.ndarray) -> str:
    if not np.any(m):
        return "dense"
    causal = np.triu(np.full((S, S), -1e9, dtype=np.float32), 1)
    if np.array_equal(m, causal):
        return "causal"
    return "masked"


def _pack_w(wT: np.ndarray, groups: int) -> np.ndarray:
    """[groups*128, M] -> [128, groups, M] partition-major contiguous."""
    g, m = groups, wT.shape[1]
    return np.ascontiguousarray(
        wT.reshape(g, 128, m).transpose(1, 0, 2)
    ).astype(NPBF16)


def make_in_maps(x, freqs_cos, freqs_sin, mask, wq, wk, wv, wo, mode):
    """Build the 8 per-core input dicts."""
    cosT = np.ascontiguousarray(np.asarray(freqs_cos, np.float32).T)  # [64, S]
    sinT = np.ascontiguousarray(np.asarray(freqs_sin, np.float32).T)
    cosT2 = np.concatenate([cosT, cosT], 0).astype(NPBF16)            # [128, S]
    # rows 0:64 get -sin (dst_e = qe*c - qo*s), rows 64:128 get +sin
    sinT2 = np.concatenate([-sinT, sinT], 0).astype(NPBF16)
    maskP = None
    if mode == "causal":
        maskP = np.ascontiguousarray(_mask4_np().transpose(1, 0, 2))  # [128,4,512]
    em = None
    if mode == "masked":
        # kernel indexes emask as [k, q]; mask is [q, k]
        em = np.exp(np.asarray(mask, np.float32).reshape(S, S)).T
        em = np.ascontiguousarray(em).astype(NPFP16)

    # permuted rows (within each head) for wq / wk
    perm_rows = (np.arange(H)[:, None] * HD + _PERM[None, :]).reshape(-1)
    wq_p = np.asarray(wq, np.float32)[perm_rows]
    wk_p = np.asarray(wk, np.float32)[perm_rows]
    wv_f = np.asarray(wv, np.float32)
    wo_f = np.asarray(wo, np.float32)
    x_f = np.asarray(x, np.float32)

    # x packed per batch: [128, NST, NC_CHUNK, ST]
    xPs = []
    for b in range(B):
        xPs.append(
            np.ascontiguousarray(
                x_f[b].reshape(NST, ST, NC_CHUNK, 128).transpose(3, 0, 2, 1)
            ).astype(NPBF16)
        )

    in_maps = []
    for core in range(NCORES):
        b, g = divmod(core, GROUPS)
        rs = slice(g * DPC, (g + 1) * DPC)
        im = {
            "xP": xPs[b],
            "wqP": _pack_w(np.ascontiguousarray(wq_p[rs].T), NC_CHUNK),
            "wkP": _pack_w(np.ascontiguousarray(wk_p[rs].T), NC_CHUNK),
            "wvP": _pack_w(np.ascontiguousarray(wv_f[rs].T), NC_CHUNK),
            "woP": _pack_w(np.ascontiguousarray(wo_f[:, rs].T), HPC),
            "cosT": cosT2,
            "sinT": sinT2,
        }
        if mode == "causal":
            im["maskP"] = maskP
        if mode == "masked":
            im["emask"] = em
        in_maps.append(im)
    return in_maps


def assemble(results) -> np.ndarray:
    out = np.zeros((B, S, D), dtype=np.float32)
    for core in range(NCORES):
        b = core // GROUPS
        out[b] += np.asarray(results[core]["out"], dtype=np.float32)
    return out


def kernel(x, freqs_cos, freqs_sin, mask, wq, wk, wv, wo, **run_kwargs):
    mode = _classify_mask(np.asarray(mask, np.float32).reshape(S, S))
    nc = _get_program(mode)
    in_maps = make_in_maps(x, freqs_cos, freqs_sin, mask, wq, wk, wv, wo, mode)
    res = run_bass_kernel_spmd(nc, in_maps, core_ids=list(range(NCORES)), **run_kwargs)
    out = assemble(res.results)
    kernel.last_results = res
    return out



# revision 29
# speedup vs baseline: 1.3889x; 1.1847x over previous
"""Trainium2 Bass kernel for causal multi-head attention with RoPE.

Problem: B=2, S=2048, D=2048, H=16 heads of HD=128.
  q/k/v = x @ w{q,k,v}.T ; RoPE(q,k) ; causal softmax(q k^T/sqrt(HD)) @ v ; @ wo.T

Sharding (8 cores): batch (2) x head-group (4 groups of 4 heads).
Each core: full projections for its 4 heads on its batch, attention, and a
partial output projection (row-shard of wo). Host sums the 4 partials per batch.

Device-side layout tricks (all host-prepared, free at HW time):
  - All streamed tensors are packed partition-major on the host so each
    load is a few large DMAs with fully contiguous per-partition lines
    (128KB chunk sprays measured only ~216 GB/s and convoyed the queues).
  - scores are computed transposed [k,q]: softmax sums via ones-matmul, AV
    yields oT [d,q] whose 128-col slices are exactly the out-proj lhsT.
  - RoPE pairs are pre-permuted into rotate-half layout (even dims in
    partitions 0:64, odd in 64:128) by permuting wq/wk rows on the host.
  - causal masking: upper-triangle k-tiles are skipped entirely; the 4
    diagonal-crossing tile shapes use precomputed 0/1 multiplicative masks.
  - output partials are written bf16 (host accumulates in fp32).

Scheduling notes (hard-won):
  - Q-proj stays in phase B as attention filler. Moving all projections
    into a dense phase A reliably trips the chip into its 2.0GHz P0 power
    state (+20% on every matmul) — measured 399us vs 329us.
  - Rope half-swap DMAs ride the sync queue; gpsimd only does the softmax
    norm partition_broadcasts (sharing one queue convoyed them).
"""

import sys

sys.path.insert(0, "/opt/trn_rl_repo")

from contextlib import ExitStack

import numpy as np
import ml_dtypes

import concourse.bass as bass
import concourse.tile as tile
from concourse import bacc, mybir
from concourse.bass_utils import run_bass_kernel_spmd

B, S, D, H = 2, 2048, 2048, 16
HD, HD2 = 128, 64
NCORES = 8
HPC = 4              # heads per core
DPC = HPC * HD       # 512
GROUPS = H // HPC    # 4 head-groups (x 2 batches = 8 cores)
SCALE = 1.0 / float(np.sqrt(HD))

ST = 512             # q-tile width (free dim of most matmuls)
NST = S // ST        # 4
KT = 128             # k-tile height (partition dim of score tiles)
NKT = S // KT        # 16
NC_CHUNK = D // 128  # 16 contraction chunks for projections

BF16 = mybir.dt.bfloat16
FP16 = mybir.dt.float16
F32 = mybir.dt.float32
NPBF16 = ml_dtypes.bfloat16
NPFP16 = np.float16
LN_INV512 = float(np.log(1.0 / 512.0))  # exp bias so fp16 sums can't overflow

EXP_FN = mybir.ActivationFunctionType.Exp


def build_program(mode: str):
    """mode: 'causal' (skip upper tiles, diag masks), 'dense' (no mask),
    'masked' (multiply every exp tile by a streamed exp(mask) tile)."""
    assert mode in ("causal", "dense", "masked")
    nc = bacc.Bacc(
        "TRN2",
        target_bir_lowering=False,
        debug=False,
        enable_asserts=False,
        num_devices=NCORES,
    )
    xP = nc.dram_tensor("xP", [128, NST, NC_CHUNK, ST], BF16, kind="ExternalInput").ap()
    wqP = nc.dram_tensor("wqP", [128, NC_CHUNK, DPC], BF16, kind="ExternalInput").ap()
    wkP = nc.dram_tensor("wkP", [128, NC_CHUNK, DPC], BF16, kind="ExternalInput").ap()
    wvP = nc.dram_tensor("wvP", [128, NC_CHUNK, DPC], BF16, kind="ExternalInput").ap()
    woP = nc.dram_tensor("woP", [128, HPC, D], BF16, kind="ExternalInput").ap()
    cosT = nc.dram_tensor("cosT", [HD, S], BF16, kind="ExternalInput").ap()
    sinT = nc.dram_tensor("sinT", [HD, S], BF16, kind="ExternalInput").ap()
    maskP = emask = None
    if mode == "causal":
        maskP = nc.dram_tensor("maskP", [KT, 4, ST], FP16, kind="ExternalInput").ap()
    if mode == "masked":
        emask = nc.dram_tensor("emask", [S, S], FP16, kind="ExternalInput").ap()
    out = nc.dram_tensor("out", [S, D], BF16, kind="ExternalOutput").ap()

    with tile.TileContext(nc) as tc, ExitStack() as ctx:
        _body(ctx, tc, mode, xP, wqP, wkP, wvP, woP, cosT, sinT, maskP, emask, out)
    nc.compile()
    return nc


def _body(ctx, tc, mode, xP, wqP, wkP, wvP, woP, cosT, sinT, maskP, emask, out):
    nc = tc.nc
    resid = ctx.enter_context(tc.tile_pool(name="resid", bufs=1))
    xpool = ctx.enter_context(tc.tile_pool(name="xpool", bufs=2))
    psum = ctx.enter_context(tc.tile_pool(name="psum", bufs=1, space="PSUM"))
    tmp = ctx.enter_context(tc.tile_pool(name="tmp", bufs=1))

    # ---- resident weights / constants ----
    # Packed layouts: 4-chunk groups = 4KB contiguous per partition line.
    wq_sb = resid.tile([128, NC_CHUNK, DPC], BF16, name="wq_sb")
    wk_sb = resid.tile([128, NC_CHUNK, DPC], BF16, name="wk_sb")
    wv_sb = resid.tile([128, NC_CHUNK, DPC], BF16, name="wv_sb")
    wo_sb = resid.tile([128, HPC, D], BF16, name="wo_sb")
    cos_sb = resid.tile([128, S], BF16, name="cos_sb")
    sin_sb = resid.tile([128, S], BF16, name="sin_sb")
    mask_sb = None

    x_tiles = {}

    def load_x(st, eng):
        t = xpool.tile([128, NC_CHUNK, ST], BF16, tag="x", bufs=2, name="x_sb")
        for g in range(4):
            eng.dma_start(out=t[:, 4 * g:4 * g + 4, :], in_=xP[:, st, 4 * g:4 * g + 4, :])
        x_tiles[st] = t
        return t

    # Startup: the scalar HWDGE ring lags ~3.5us at kernel start (the ACT
    # table-load DMA occupies it), so the startup-critical wk and x(1)
    # loads ride the sync + vector rings in parallel, interleaved in
    # progressive groups — the first K-proj matmul needs only chunk 0.
    # HAM warmup: the PE clock gate is cold (1.2GHz) until ~3.4us of
    # sustained activity; dummy matmuls during the startup DMA wait
    # flip it to full rate before the first real matmul retires.
    warm_sb = resid.tile([128, 128], BF16, name="warm_sb")
    nc.vector.memset(warm_sb, 0.0)
    for _ in range(28):
        pw = psum.tile([128, 128], F32, tag="pv", bufs=4, name="ps_warm")
        nc.tensor.matmul(pw, warm_sb, warm_sb, start=True, stop=True)

    # Startup rings: the sync HWDGE carries the startup-critical
    # interleaved (wk c, x1 c) pairs for c0-9; the scalar ring (lags
    # ~3.5us behind the ACT table load) takes c10-15 — those aren't
    # consumed until ~12us in, by which point scalar has caught up.
    # gpsimd DMA is a software DGE and measured far too slow for bulk
    # weights (wk-on-gpsimd cost ~24us of early PE stalls).
    # K-proj consumes (wk c, x c) pairs at ~250GB/s — more than one
    # ~216GB/s HWDGE ring sustains — so chunk pairs ALTERNATE between the
    # sync and scalar rings (each then needs only ~125GB/s). scalar's
    # ~3.5us start lag only affects chunk 1, consumed at ~8us. cos/sin
    # feed the first ropes (~22us), wv the first V-proj (~24us) — split
    # across both rings after the chunks; mask (read ~150us) goes last.
    x1 = xpool.tile([128, NC_CHUNK, ST], BF16, tag="x", bufs=2, name="x_sb")
    for c in range(NC_CHUNK):
        eng = nc.sync if c % 2 == 0 else nc.scalar
        eng.dma_start(out=wk_sb[:, c:c + 1, :], in_=wkP[:, c:c + 1, :])
        eng.dma_start(out=x1[:, c:c + 1, :], in_=xP[:, 1, c:c + 1, :])
    x_tiles[1] = x1
    nc.sync.dma_start(out=cos_sb, in_=cosT)
    nc.scalar.dma_start(out=sin_sb, in_=sinT)
    for g in range(4):
        eng = nc.sync if g % 2 == 0 else nc.scalar
        eng.dma_start(out=wv_sb[:, 4 * g:4 * g + 4, :], in_=wvP[:, 4 * g:4 * g + 4, :])
    if mode == "causal":
        mask_sb = resid.tile([128, 4, ST], FP16, name="mask_sb")
        nc.scalar.dma_start(out=mask_sb, in_=maskP)

    ones_sb = resid.tile([128, 1], FP16, name="ones_sb")
    nc.vector.memset(ones_sb, 1.0)
    ebias_sb = resid.tile([128, 1], F32, name="ebias_sb")
    nc.vector.memset(ebias_sb, LN_INV512)
    onesf_sb = resid.tile([1, 128], FP16, name="onesf_sb")
    nc.vector.memset(onesf_sb, 1.0)
    # Preload the ACT exp table during the startup DMA wait: the first
    # ACTIVATE(Exp) otherwise pays a ~2.7us lazy table load right at
    # attention start (and the PE gap it causes trips a HAM re-throttle).
    twarm_sb = resid.tile([128, 1], F32, name="twarm_sb")
    nc.scalar.activation(twarm_sb, ebias_sb, func=EXP_FN, scale=1.0)

    # ---- resident activations ----
    qT_sb = resid.tile([128, HPC, S], BF16, name="qT_sb")   # [d, h, q-pos]
    kT_sb = resid.tile([128, HPC, S], BF16, name="kT_sb")   # [d, h, k-pos]
    v_sb = resid.tile([128, NKT, DPC], FP16, name="v_sb")   # [k-pos%128, k-tile, hd]
    oT_sb = resid.tile([128, HPC, S], BF16, name="oT_sb")   # [d, h, q-pos]

    # RoPE (rotate-half layout): dst = t*[c;c] + swap(t)*[-s;s].
    # Engines can't cross partitions, so the half-swap is a DMA.
    # copy_eng: scalar in phase A (ACT idle), vector during attention
    # (ACT is exp-saturated there; DVE has slack).
    def rope(ps, dstT, h, ssl, copy_eng=None):
        stg = tmp.tile([128, ST], BF16, tag="stg", bufs=3, name="stg")
        if copy_eng is nc.vector:
            nc.vector.tensor_copy(stg, ps)
        else:
            nc.scalar.copy(stg, ps)
        swp = tmp.tile([128, ST], BF16, tag="swp", bufs=3, name="swp")
        nc.sync.dma_start(out=swp[0:64, :], in_=stg[64:128, :])
        nc.sync.dma_start(out=swp[64:128, :], in_=stg[0:64, :])
        t1 = tmp.tile([128, ST], BF16, tag="t1", bufs=2, name="t1")
        t2 = tmp.tile([128, ST], BF16, tag="t2", bufs=2, name="t2")
        nc.vector.tensor_mul(t1, stg, cos_sb[:, ssl])
        nc.vector.tensor_mul(t2, swp, sin_sb[:, ssl])
        nc.vector.tensor_add(dstT[:, h, ssl], t1, t2)

    # ========== Phase A: K projection + RoPE, V projection ==========
    # K and V are needed in full before any attention; Q is deferred so its
    # matmuls can hide the exp-heavy attention phase.
    # st=0 goes last so its x tile is still live for qproj(0) in phase B.
    order = (1, 2, 3, 0)
    for idx, st in enumerate(order):
        ssl = slice(st * ST, (st + 1) * ST)
        x_sb = x_tiles[st]
        if idx + 1 < len(order):
            # x(2) on sync (free after startup); x(3)/x(0) on the scalar
            # ring so they don't queue behind the startup interleave
            load_x(order[idx + 1], nc.sync if idx == 0 else nc.scalar)
        psks = []
        for h in range(HPC):
            psks.append(psum.tile([128, ST], F32, tag="pj", bufs=4, name="ps_k"))
        for c in range(NC_CHUNK):
            for h in range(HPC):
                nc.tensor.matmul(
                    psks[h], wk_sb[:, c, h * HD:(h + 1) * HD], x_sb[:, c, :],
                    start=(c == 0), stop=(c == NC_CHUNK - 1),
                )
        for h in range(HPC):
            rope(psks[h], kT_sb, h, ssl)
        for s4 in range(ST // 128):
            pv = psum.tile([128, DPC], F32, tag="pv", bufs=4, name="ps_pv")
            for c in range(NC_CHUNK):
                nc.tensor.matmul(
                    pv, x_sb[:, c, s4 * 128:(s4 + 1) * 128], wv_sb[:, c, :],
                    start=(c == 0), stop=(c == NC_CHUNK - 1),
                )
            nc.vector.tensor_copy(v_sb[:, st * 4 + s4, :], pv)
        if st != 0:
            # only the last-loaded tile (st=0) survives the 2-deep pool
            del x_tiles[st]

    # wq/wo load last on the scalar ring: first needed ~115us/170us in, and
    # emitting them here keeps their completions behind the x prefetches.
    for g in range(4):
        nc.scalar.dma_start(out=wq_sb[:, 4 * g:4 * g + 4, :], in_=wqP[:, 4 * g:4 * g + 4, :])
    nc.scalar.dma_start(out=wo_sb[:, 0:2, :], in_=woP[:, 0:2, :])
    nc.scalar.dma_start(out=wo_sb[:, 2:4, :], in_=woP[:, 2:4, :])

    # ========== Phase B: Q projection interleaved with attention ==========
    # Generators emit filler matmuls (Q-proj / out-proj) that the attention
    # loop interleaves between exp-dependent tiles, keeping PE fed while the
    # scalar engine computes exp.
    def qproj_gen(st):
        ssl = slice(st * ST, (st + 1) * ST)
        if st in x_tiles:
            x_sb = x_tiles.pop(st)
        else:
            x_sb = load_x(st, nc.sync)

        def inner():
            for h in range(HPC):
                psq = psum.tile([128, ST], F32, tag="pj", bufs=4, name="ps_q")
                for c in range(NC_CHUNK):
                    nc.tensor.matmul(
                        psq, wq_sb[:, c, h * HD:(h + 1) * HD], x_sb[:, c, :],
                        start=(c == 0), stop=(c == NC_CHUNK - 1),
                    )
                    yield
                rope(psq, qT_sb, h, ssl, copy_eng=nc.vector)
                yield

        return inner()

    # Softmax norm, split in two stages (stage1: sum matmul + reciprocal;
    # stage2: broadcast + oT multiply). accs is 1-2 partial accumulators
    # (DVE + gpsimd chains); the psm matmul accumulates over them.
    def norm_stage1(h, qt, po, accs):
        psm = psum.tile([1, ST], F32, tag="pv", bufs=4, name="ps_sum")
        for i, a in enumerate(accs):
            nc.tensor.matmul(psm, ones_sb, a,
                             start=(i == 0), stop=(i == len(accs) - 1))
        r_row = tmp.tile([1, ST], F32, tag="r", bufs=2, name="r_row")
        nc.vector.reciprocal_approx_fast(r_row, psm)
        return (h, qt, po, r_row)

    def norm_stage2(st1, use_pe=False):
        h, qt, po, r_row = st1
        qsl = slice(qt * ST, (qt + 1) * ST)
        rb_sb = tmp.tile([128, ST], F32, tag="rb", bufs=2, name="rb_sb")
        if use_pe:
            # final norms sit on the critical tail: PE broadcast is faster
            # than gpsimd's ~3.3us partition_broadcast
            r16 = tmp.tile([1, ST], FP16, tag="r16", bufs=1, name="r16")
            nc.vector.tensor_copy(r16, r_row)
            prb = psum.tile([128, ST], F32, tag="pv", bufs=4, name="ps_rb")
            nc.tensor.matmul(prb, onesf_sb, r16, start=True, stop=True)
            nc.scalar.copy(rb_sb, prb)
        else:
            nc.gpsimd.partition_broadcast(rb_sb, r_row)
        nc.vector.tensor_mul(oT_sb[:, h, qsl], po, rb_sb)

    def emit_norm(h, qt, po, acc, use_pe=False):
        norm_stage2(norm_stage1(h, qt, po, acc), use_pe=use_pe)

    def outproj_gen(qt, wide=False):
        # wide=True (tail only): 4 PSUM tiles from the now-idle pv tag join
        # the rotation so copies never gate the matmul stream.
        def inner():
            for s128 in range(qt * 4, (qt + 1) * 4):
                if wide:
                    pouts = [
                        psum.tile([128, ST], F32, tag="pj", bufs=4, name="ps_out"),
                        psum.tile([128, ST], F32, tag="pj", bufs=4, name="ps_out"),
                        psum.tile([128, ST], F32, tag="pv", bufs=4, name="ps_out"),
                        psum.tile([128, ST], F32, tag="pv", bufs=4, name="ps_out"),
                    ]
                    for h in range(HPC):
                        for j in range(4):
                            nc.tensor.matmul(
                                pouts[j],
                                oT_sb[:, h, s128 * 128:(s128 + 1) * 128],
                                wo_sb[:, h, j * ST:(j + 1) * ST],
                                start=(h == 0), stop=(h == HPC - 1),
                            )
                        yield
                    for j in range(4):
                        o_sb = tmp.tile([128, ST], BF16, tag="osb", bufs=4, name="o_sb")
                        if j % 2 == 0:
                            nc.vector.tensor_copy(o_sb, pouts[j])
                        else:
                            nc.scalar.copy(o_sb, pouts[j])
                        eng = nc.sync if j % 2 == 0 else nc.scalar
                        eng.dma_start(
                            out=out[s128 * 128:(s128 + 1) * 128, j * ST:(j + 1) * ST],
                            in_=o_sb,
                        )
                        yield
                else:
                    for jp in range(2):
                        pouts = []
                        for jj in range(2):
                            pj_ = psum.tile(
                                [128, ST], F32, tag="pj", bufs=4, name="ps_out"
                            )
                            pouts.append(pj_)
                        for h in range(HPC):
                            for jj in range(2):
                                j = 2 * jp + jj
                                nc.tensor.matmul(
                                    pouts[jj],
                                    oT_sb[:, h, s128 * 128:(s128 + 1) * 128],
                                    wo_sb[:, h, j * ST:(j + 1) * ST],
                                    start=(h == 0), stop=(h == HPC - 1),
                                )
                            yield
                        for jj in range(2):
                            j = 2 * jp + jj
                            o_sb = tmp.tile([128, ST], BF16, tag="osb", bufs=4, name="o_sb")
                            if j % 2 == 0:
                                nc.vector.tensor_copy(o_sb, pouts[jj])
                            else:
                                nc.scalar.copy(o_sb, pouts[jj])
                            nc.sync.dma_start(
                                out=out[s128 * 128:(s128 + 1) * 128, j * ST:(j + 1) * ST],
                                in_=o_sb,
                            )
                        yield

        return inner()

    def drain(gen):
        for _ in gen:
            pass

    pend = [None]
    pull_acc = [0.0]

    def attn(qt, filler, pulls):
        for h in range(HPC):
            nkt = 4 * (qt + 1) if mode == "causal" else NKT
            po = psum.tile([128, ST], F32, tag="pj", bufs=4, name="ps_po")
            # (gpsimd acc-split was tried and reverted: gpsimd tensor ops
            # measure 1.1-2us each — the odd-kt chain lagged the ~900ns
            # tile cadence and the norm matmul stalled the in-order PE
            # queue 3-5us per head.)
            acc = tmp.tile([128, ST], FP16, tag="acc", bufs=2, name="acc")

            def emit_av(kt, q0, e_sb, po=po, h=h, nkt=nkt):
                nc.tensor.matmul(
                    po[:, q0:],
                    v_sb[:, kt, h * HD:(h + 1) * HD],
                    e_sb[:, q0:],
                    start=(kt == 0), stop=(kt == nkt - 1),
                )

            prev_av = None
            for kt in range(nkt):
                di = kt - 4 * qt
                # diagonal tiles only have valid queries at columns >= di*KT
                q0 = di * KT if (mode == "causal" and di >= 0) else 0
                qsl = slice(qt * ST + q0, (qt + 1) * ST)
                pss = psum.tile([128, ST], F32, tag="pv", bufs=4, name="ps_s")
                nc.tensor.matmul(
                    pss[:, q0:],
                    kT_sb[:, h, kt * KT:(kt + 1) * KT],
                    qT_sb[:, h, qsl],
                    start=True, stop=True,
                )
                e_sb = tmp.tile([128, ST], FP16, tag="e", bufs=7, name="e_sb")
                nc.scalar.activation(
                    e_sb[:, q0:], pss[:, q0:], func=EXP_FN,
                    scale=SCALE, bias=ebias_sb,
                )
                if mode == "causal":
                    if di >= 0:
                        nc.vector.tensor_mul(
                            e_sb[:, q0:], e_sb[:, q0:], mask_sb[:, di, q0:]
                        )
                elif mode == "masked":
                    m_sb = tmp.tile([128, ST], FP16, tag="m", bufs=4, name="m_sb")
                    nc.sync.dma_start(
                        out=m_sb, in_=emask[kt * KT:(kt + 1) * KT, qsl]
                    )
                    nc.vector.tensor_mul(e_sb, e_sb, m_sb)
                if kt == 0:
                    nc.vector.tensor_copy(acc, e_sb)
                else:
                    nc.vector.tensor_add(acc[:, q0:], acc[:, q0:], e_sb[:, q0:])
                if prev_av is not None:
                    emit_av(*prev_av)
                prev_av = (kt, q0, e_sb)
                if kt == 0 and pend[0] is not None:
                    # Flush the previous head's deferred norm AFTER this
                    # head's first score tile: the psm matmul then never
                    # heads the in-order PE queue while the previous
                    # head's DVE acc chain is still draining.
                    emit_norm(*pend[0])
                    pend[0] = None
                pull_acc[0] += pulls
                while pull_acc[0] >= 1.0:
                    pull_acc[0] -= 1.0
                    if next(filler, "end") == "end":
                        pull_acc[0] = 0.0
                        break
            emit_av(*prev_av)
            pend[0] = (h, qt, po, [acc])

    # pulls is yields-per-score-tile; sized so each generator lasts its
    # whole phase (dry filler = PE waits on the exp chain): yield counts
    # qproj=68, outproj=40 vs tiles 16/32/48/64.
    drain(qproj_gen(0))
    filler = qproj_gen(1)
    attn(0, filler, 4)
    drain(filler)
    filler = qproj_gen(2)
    attn(1, filler, 2)
    drain(filler)
    filler = outproj_gen(0)
    f2 = qproj_gen(3)
    import itertools
    filler = itertools.chain(f2, filler)
    attn(2, filler, 2)
    drain(filler)
    filler = itertools.chain(outproj_gen(1), outproj_gen(2))
    attn(3, filler, 1.25)
    drain(filler)
    emit_norm(*pend[0], use_pe=True)
    drain(outproj_gen(3, wide=True))


# ---------------------------------------------------------------------------
# Host side
# ---------------------------------------------------------------------------

_PROGRAMS: dict = {}


def _get_program(mode: str):
    if mode not in _PROGRAMS:
        _PROGRAMS[mode] = build_program(mode)
    return _PROGRAMS[mode]


_PERM = np.concatenate([np.arange(0, HD, 2), np.arange(1, HD, 2)])  # rotate-half


def _mask4_np() -> np.ndarray:
    m = np.zeros((4, KT, ST), dtype=np.float32)
    p = np.arange(KT)[:, None]
    qf = np.arange(ST)[None, :]
    for di in range(4):
        m[di] = (qf >= di * KT + p).astype(np.float32)
    return m.astype(NPFP16)


def _classify_mask(m: np.ndarray) -> str:
    if not np.any(m):
        return "dense"
    causal = np.triu(np.full((S, S), -1e9, dtype=np.float32), 1)
    if np.array_equal(m, causal):
        return "causal"
    return "masked"


def _pack_w(wT: np.ndarray, groups: int) -> np.ndarray:
    """[groups*128, M] -> [128, groups, M] partition-major contiguous."""
    g, m = groups, wT.shape[1]
    return np.ascontiguousarray(
        wT.reshape(g, 128, m).transpose(1, 0, 2)
    ).astype(NPBF16)


def make_in_maps(x, freqs_cos, freqs_sin, mask, wq, wk, wv, wo, mode):
    """Build the 8 per-core input dicts."""
    cosT = np.ascontiguousarray(np.asarray(freqs_cos, np.float32).T)  # [64, S]
    sinT = np.ascontiguousarray(np.asarray(freqs_sin, np.float32).T)
    cosT2 = np.concatenate([cosT, cosT], 0).astype(NPBF16)            # [128, S]
    # rows 0:64 get -sin (dst_e = qe*c - qo*s), rows 64:128 get +sin
    sinT2 = np.concatenate([-sinT, sinT], 0).astype(NPBF16)
    maskP = None
    if mode == "causal":
        maskP = np.ascontiguousarray(_mask4_np().transpose(1, 0, 2))  # [128,4,512]
    em = None
    if mode == "masked":
        # kernel indexes emask as [k, q]; mask is [q, k]
        em = np.exp(np.asarray(mask, np.float32).reshape(S, S)).T
        em = np.ascontiguousarray(em).astype(NPFP16)

    # permuted rows (within each head) for wq / wk
    perm_rows = (np.arange(H)[:, None] * HD + _PERM[None, :]).reshape(-1)
    wq_p = np.asarray(wq, np.float32)[perm_rows]
    wk_p = np.asarray(wk, np.float32)[perm_rows]
    wv_f = np.asarray(wv, np.float32)
    wo_f = np.asarray(wo, np.float32)
    x_f = np.asarray(x, np.float32)

    # x packed per batch: [128, NST, NC_CHUNK, ST]
    xPs = []
    for b in range(B):
        xPs.append(
            np.ascontiguousarray(
                x_f[b].reshape(NST, ST, NC_CHUNK, 128).transpose(3, 0, 2, 1)
            ).astype(NPBF16)
        )

    in_maps = []
    for core in range(NCORES):
        b, g = divmod(core, GROUPS)
        rs = slice(g * DPC, (g + 1) * DPC)
        im = {
            "xP": xPs[b],
            "wqP": _pack_w(np.ascontiguousarray(wq_p[rs].T), NC_CHUNK),
            "wkP": _pack_w(np.ascontiguousarray(wk_p[rs].T), NC_CHUNK),
            "wvP": _pack_w(np.ascontiguousarray(wv_f[rs].T), NC_CHUNK),
            "woP": _pack_w(np.ascontiguousarray(wo_f[:, rs].T), HPC),
            "cosT": cosT2,
            "sinT": sinT2,
        }
        if mode == "causal":
            im["maskP"] = maskP
        if mode == "masked":
            im["emask"] = em
        in_maps.append(im)
    return in_maps


def assemble(results) -> np.ndarray:
    out = np.zeros((B, S, D), dtype=np.float32)
    for core in range(NCORES):
        b = core // GROUPS
        out[b] += np.asarray(results[core]["out"], dtype=np.float32)
    return out


def kernel(x, freqs_cos, freqs_sin, mask, wq, wk, wv, wo, **run_kwargs):
    mode = _classify_mask(np.asarray(mask, np.float32).reshape(S, S))
    nc = _get_program(mode)
    in_maps = make_in_maps(x, freqs_cos, freqs_sin, mask, wq, wk, wv, wo, mode)
    res = run_bass_kernel_spmd(nc, in_maps, core_ids=list(range(NCORES)), **run_kwargs)
    out = assemble(res.results)
    kernel.last_results = res
    return out



# revision 34
# speedup vs baseline: 1.4075x; 1.0134x over previous
"""Trainium2 Bass kernel for causal multi-head attention with RoPE.

Problem: B=2, S=2048, D=2048, H=16 heads of HD=128.
  q/k/v = x @ w{q,k,v}.T ; RoPE(q,k) ; causal softmax(q k^T/sqrt(HD)) @ v ; @ wo.T

Sharding (8 cores): batch (2) x head-group (4 groups of 4 heads).
Each core: full projections for its 4 heads on its batch, attention, and a
partial output projection (row-shard of wo). Host sums the 4 partials per batch.

Device-side layout tricks (all host-prepared, free at HW time):
  - All streamed tensors are packed partition-major on the host so each
    load is a few large DMAs with fully contiguous per-partition lines
    (128KB chunk sprays measured only ~216 GB/s and convoyed the queues).
  - scores are computed transposed [k,q]: softmax sums via ones-matmul, AV
    yields oT [d,q] whose 128-col slices are exactly the out-proj lhsT.
  - RoPE pairs are pre-permuted into rotate-half layout (even dims in
    partitions 0:64, odd in 64:128) by permuting wq/wk rows on the host.
  - causal masking: upper-triangle k-tiles are skipped entirely; the 4
    diagonal-crossing tile shapes use precomputed 0/1 multiplicative masks.
  - output partials are written bf16 (host accumulates in fp32).

Scheduling notes (hard-won):
  - The chip lands in either a 2.4GHz or 2.0GHz (P0) clock regime per
    run, ambient, not kernel-controlled: identical kernels measure 216ns
    vs 259ns issue gaps for 512-wide matmuls (~13% wall swing). Compare
    runs by median matmul gap, not raw ns.
  - Q-proj/out-proj matmuls interleave into attention as fillers (the
    per-tile exp chain leaves the PE ~400ns short per score tile);
    pulls are sized so generators last their whole phase.
  - HAM warmup dummy matmuls + an exp-table preload at kernel start
    avoid the cold 1.2GHz ramp and a ~2.7us first-exp stall.
  - DMA descriptors cost ~650ns engine-issue each; weight streams use
    doubling-size groups alternated across the sync/scalar rings.
  - gpsimd is slow (software DGE DMA; ~1.1-2us tensor ops) — it only
    does the softmax norm partition_broadcasts.
"""

import sys

sys.path.insert(0, "/opt/trn_rl_repo")

from contextlib import ExitStack

import numpy as np
import ml_dtypes

import concourse.bass as bass
import concourse.tile as tile
from concourse import bacc, mybir
from concourse.bass_utils import run_bass_kernel_spmd

B, S, D, H = 2, 2048, 2048, 16
HD, HD2 = 128, 64
NCORES = 8
HPC = 4              # heads per core
DPC = HPC * HD       # 512
GROUPS = H // HPC    # 4 head-groups (x 2 batches = 8 cores)
SCALE = 1.0 / float(np.sqrt(HD))

ST = 512             # q-tile width (free dim of most matmuls)
NST = S // ST        # 4
KT = 128             # k-tile height (partition dim of score tiles)
NKT = S // KT        # 16
NC_CHUNK = D // 128  # 16 contraction chunks for projections

BF16 = mybir.dt.bfloat16
FP16 = mybir.dt.float16
F32 = mybir.dt.float32
NPBF16 = ml_dtypes.bfloat16
NPFP16 = np.float16
LN_INV512 = float(np.log(1.0 / 512.0))  # exp bias so fp16 sums can't overflow

EXP_FN = mybir.ActivationFunctionType.Exp


def build_program(mode: str):
    """mode: 'causal' (skip upper tiles, diag masks), 'dense' (no mask),
    'masked' (multiply every exp tile by a streamed exp(mask) tile)."""
    assert mode in ("causal", "dense", "masked")
    nc = bacc.Bacc(
        "TRN2",
        target_bir_lowering=False,
        debug=False,
        enable_asserts=False,
        num_devices=NCORES,
    )
    xP = nc.dram_tensor("xP", [128, NST, NC_CHUNK, ST], BF16, kind="ExternalInput").ap()
    wqP = nc.dram_tensor("wqP", [128, NC_CHUNK, DPC], BF16, kind="ExternalInput").ap()
    wkP = nc.dram_tensor("wkP", [128, NC_CHUNK, DPC], BF16, kind="ExternalInput").ap()
    wvP = nc.dram_tensor("wvP", [128, NC_CHUNK, DPC], BF16, kind="ExternalInput").ap()
    woP = nc.dram_tensor("woP", [128, HPC, D], BF16, kind="ExternalInput").ap()
    cosT = nc.dram_tensor("cosT", [HD, S], BF16, kind="ExternalInput").ap()
    sinT = nc.dram_tensor("sinT", [HD, S], BF16, kind="ExternalInput").ap()
    maskP = emask = None
    if mode == "causal":
        maskP = nc.dram_tensor("maskP", [KT, 4, ST], FP16, kind="ExternalInput").ap()
    if mode == "masked":
        emask = nc.dram_tensor("emask", [S, S], FP16, kind="ExternalInput").ap()
    out = nc.dram_tensor("out", [S, D], BF16, kind="ExternalOutput").ap()

    with tile.TileContext(nc) as tc, ExitStack() as ctx:
        _body(ctx, tc, mode, xP, wqP, wkP, wvP, woP, cosT, sinT, maskP, emask, out)
    nc.compile()
    return nc


def _body(ctx, tc, mode, xP, wqP, wkP, wvP, woP, cosT, sinT, maskP, emask, out):
    nc = tc.nc
    resid = ctx.enter_context(tc.tile_pool(name="resid", bufs=1))
    xpool = ctx.enter_context(tc.tile_pool(name="xpool", bufs=2))
    psum = ctx.enter_context(tc.tile_pool(name="psum", bufs=1, space="PSUM"))
    tmp = ctx.enter_context(tc.tile_pool(name="tmp", bufs=1))

    # ---- resident weights / constants ----
    # Packed layouts: 4-chunk groups = 4KB contiguous per partition line.
    wq_sb = resid.tile([128, NC_CHUNK, DPC], BF16, name="wq_sb")
    wk_sb = resid.tile([128, NC_CHUNK, DPC], BF16, name="wk_sb")
    wv_sb = resid.tile([128, NC_CHUNK, DPC], BF16, name="wv_sb")
    wo_sb = resid.tile([128, HPC, D], BF16, name="wo_sb")
    cos_sb = resid.tile([128, S], BF16, name="cos_sb")
    sin_sb = resid.tile([128, S], BF16, name="sin_sb")
    mask_sb = None

    x_tiles = {}

    def load_x(st, eng):
        t = xpool.tile([128, NC_CHUNK, ST], BF16, tag="x", bufs=2, name="x_sb")
        for g in range(4):
            eng.dma_start(out=t[:, 4 * g:4 * g + 4, :], in_=xP[:, st, 4 * g:4 * g + 4, :])
        x_tiles[st] = t
        return t

    # Startup: the scalar HWDGE ring lags ~3.5us at kernel start (the ACT
    # table-load DMA occupies it), so the startup-critical wk and x(1)
    # loads ride the sync + vector rings in parallel, interleaved in
    # progressive groups — the first K-proj matmul needs only chunk 0.
    # HAM warmup: the PE clock gate is cold (1.2GHz) until ~3.4us of
    # sustained activity; dummy matmuls during the startup DMA wait
    # flip it to full rate before the first real matmul retires.
    warm_sb = resid.tile([128, 128], BF16, name="warm_sb")
    nc.vector.memset(warm_sb, 0.0)
    for _ in range(10):
        pw = psum.tile([128, 128], F32, tag="pv", bufs=4, name="ps_warm")
        nc.tensor.matmul(pw, warm_sb, warm_sb, start=True, stop=True)

    # Startup rings: the sync HWDGE carries the startup-critical
    # interleaved (wk c, x1 c) pairs for c0-9; the scalar ring (lags
    # ~3.5us behind the ACT table load) takes c10-15 — those aren't
    # consumed until ~12us in, by which point scalar has caught up.
    # gpsimd DMA is a software DGE and measured far too slow for bulk
    # weights (wk-on-gpsimd cost ~24us of early PE stalls).
    # K-proj consumes (wk c, x c) pairs at ~300GB/s warm — more than one
    # ~216GB/s HWDGE ring sustains — so chunk groups alternate between
    # the sync and scalar rings. Each DMA descriptor costs ~650ns of
    # engine issue time (measured), so groups DOUBLE in size: small
    # groups up front for low first-chunk latency, big groups behind
    # to keep total descriptor count low.
    x1 = xpool.tile([128, NC_CHUNK, ST], BF16, tag="x", bufs=2, name="x_sb")
    c0 = 0
    for gi, g in enumerate((1, 1, 2, 2, 4, 6)):
        eng = nc.sync if gi % 2 == 0 else nc.scalar
        eng.dma_start(out=wk_sb[:, c0:c0 + g, :], in_=wkP[:, c0:c0 + g, :])
        eng.dma_start(out=x1[:, c0:c0 + g, :], in_=xP[:, 1, c0:c0 + g, :])
        c0 += g
    x_tiles[1] = x1
    nc.sync.dma_start(out=cos_sb, in_=cosT)
    nc.scalar.dma_start(out=sin_sb, in_=sinT)
    for g in range(4):
        eng = nc.sync if g % 2 == 0 else nc.scalar
        eng.dma_start(out=wv_sb[:, 4 * g:4 * g + 4, :], in_=wvP[:, 4 * g:4 * g + 4, :])
    if mode == "causal":
        mask_sb = resid.tile([128, 4, ST], FP16, name="mask_sb")
        nc.scalar.dma_start(out=mask_sb, in_=maskP)

    ones_sb = resid.tile([128, 1], FP16, name="ones_sb")
    nc.vector.memset(ones_sb, 1.0)
    ebias_sb = resid.tile([128, 1], F32, name="ebias_sb")
    nc.vector.memset(ebias_sb, LN_INV512)
    onesf_sb = resid.tile([1, 128], FP16, name="onesf_sb")
    nc.vector.memset(onesf_sb, 1.0)
    # Preload the ACT exp table during the startup DMA wait: the first
    # ACTIVATE(Exp) otherwise pays a ~2.7us lazy table load right at
    # attention start (and the PE gap it causes trips a HAM re-throttle).
    twarm_sb = resid.tile([128, 1], F32, name="twarm_sb")
    nc.scalar.activation(twarm_sb, ebias_sb, func=EXP_FN, scale=1.0)

    # ---- resident activations ----
    qT_sb = resid.tile([128, HPC, S], BF16, name="qT_sb")   # [d, h, q-pos]
    kT_sb = resid.tile([128, HPC, S], BF16, name="kT_sb")   # [d, h, k-pos]
    v_sb = resid.tile([128, NKT, DPC], FP16, name="v_sb")   # [k-pos%128, k-tile, hd]
    oT_sb = resid.tile([128, HPC, S], BF16, name="oT_sb")   # [d, h, q-pos]

    # RoPE (rotate-half layout): dst = t*[c;c] + swap(t)*[-s;s].
    # Engines can't cross partitions, so the half-swap is a DMA.
    # copy_eng: scalar in phase A (ACT idle), vector during attention
    # (ACT is exp-saturated there; DVE has slack).
    def rope(ps, dstT, h, ssl, copy_eng=None):
        stg = tmp.tile([128, ST], BF16, tag="stg", bufs=3, name="stg")
        if copy_eng is nc.vector:
            nc.vector.tensor_copy(stg, ps)
        else:
            nc.scalar.copy(stg, ps)
        swp = tmp.tile([128, ST], BF16, tag="swp", bufs=3, name="swp")
        nc.sync.dma_start(out=swp[0:64, :], in_=stg[64:128, :])
        nc.sync.dma_start(out=swp[64:128, :], in_=stg[0:64, :])
        t1 = tmp.tile([128, ST], BF16, tag="t1", bufs=2, name="t1")
        t2 = tmp.tile([128, ST], BF16, tag="t2", bufs=2, name="t2")
        nc.vector.tensor_mul(t1, stg, cos_sb[:, ssl])
        nc.vector.tensor_mul(t2, swp, sin_sb[:, ssl])
        nc.vector.tensor_add(dstT[:, h, ssl], t1, t2)

    # ========== Phase A: K projection + RoPE, V projection ==========
    # K and V are needed in full before any attention; Q is deferred so its
    # matmuls can hide the exp-heavy attention phase.
    # st=0 goes last so its x tile is still live for qproj(0) in phase B.
    order = (1, 2, 3, 0)
    for idx, st in enumerate(order):
        ssl = slice(st * ST, (st + 1) * ST)
        x_sb = x_tiles[st]
        if idx + 1 < len(order):
            # x(2) on sync (free after startup); x(3)/x(0) on the scalar
            # ring so they don't queue behind the startup interleave
            load_x(order[idx + 1], nc.sync if idx == 0 else nc.scalar)
        psks = []
        for h in range(HPC):
            psks.append(psum.tile([128, ST], F32, tag="pj", bufs=4, name="ps_k"))
        for c in range(NC_CHUNK):
            for h in range(HPC):
                nc.tensor.matmul(
                    psks[h], wk_sb[:, c, h * HD:(h + 1) * HD], x_sb[:, c, :],
                    start=(c == 0), stop=(c == NC_CHUNK - 1),
                )
        # V-proj matmuls are emitted BEFORE the ropes: they depend only on
        # x/wv, so the PE flows K->V seamlessly while the rope chains
        # (DVE copy + sync swap DMA + DVE muls) drain the psks banks in
        # the background. Rope copies ride DVE in phase A — the scalar
        # queue is still issuing weight-stream DMA descriptors here.
        for s4 in range(ST // 128):
            pv = psum.tile([128, DPC], F32, tag="pv", bufs=4, name="ps_pv")
            for c in range(NC_CHUNK):
                nc.tensor.matmul(
                    pv, x_sb[:, c, s4 * 128:(s4 + 1) * 128], wv_sb[:, c, :],
                    start=(c == 0), stop=(c == NC_CHUNK - 1),
                )
            nc.vector.tensor_copy(v_sb[:, st * 4 + s4, :], pv)
            if s4 < HPC:
                rope(psks[s4], kT_sb, s4, ssl, copy_eng=nc.vector)
        if st != 0:
            # only the last-loaded tile (st=0) survives the 2-deep pool
            del x_tiles[st]

    # wq/wo load last on the scalar ring: first needed ~115us/170us in, and
    # emitting them here keeps their completions behind the x prefetches.
    for g in range(4):
        nc.scalar.dma_start(out=wq_sb[:, 4 * g:4 * g + 4, :], in_=wqP[:, 4 * g:4 * g + 4, :])
    nc.scalar.dma_start(out=wo_sb[:, 0:2, :], in_=woP[:, 0:2, :])
    nc.scalar.dma_start(out=wo_sb[:, 2:4, :], in_=woP[:, 2:4, :])

    # ========== Phase B: Q projection interleaved with attention ==========
    # Generators emit filler matmuls (Q-proj / out-proj) that the attention
    # loop interleaves between exp-dependent tiles, keeping PE fed while the
    # scalar engine computes exp.
    def qproj_gen(st):
        ssl = slice(st * ST, (st + 1) * ST)
        if st in x_tiles:
            x_sb = x_tiles.pop(st)
        else:
            x_sb = load_x(st, nc.sync)

        def inner():
            for h in range(HPC):
                psq = psum.tile([128, ST], F32, tag="pj", bufs=4, name="ps_q")
                for c in range(NC_CHUNK):
                    nc.tensor.matmul(
                        psq, wq_sb[:, c, h * HD:(h + 1) * HD], x_sb[:, c, :],
                        start=(c == 0), stop=(c == NC_CHUNK - 1),
                    )
                    yield
                rope(psq, qT_sb, h, ssl, copy_eng=nc.vector)
                yield

        return inner()

    # Softmax norm, split in two stages (stage1: sum matmul + reciprocal;
    # stage2: broadcast + oT multiply). accs is 1-2 partial accumulators
    # (DVE + gpsimd chains); the psm matmul accumulates over them.
    def norm_stage1(h, qt, po, accs):
        psm = psum.tile([1, ST], F32, tag="pv", bufs=4, name="ps_sum")
        for i, a in enumerate(accs):
            nc.tensor.matmul(psm, ones_sb, a,
                             start=(i == 0), stop=(i == len(accs) - 1))
        r_row = tmp.tile([1, ST], F32, tag="r", bufs=2, name="r_row")
        nc.vector.reciprocal_approx_fast(r_row, psm)
        return (h, qt, po, r_row)

    def norm_stage2(st1, use_pe=False):
        h, qt, po, r_row = st1
        qsl = slice(qt * ST, (qt + 1) * ST)
        rb_sb = tmp.tile([128, ST], F32, tag="rb", bufs=2, name="rb_sb")
        if use_pe:
            # final norms sit on the critical tail: PE broadcast is faster
            # than gpsimd's ~3.3us partition_broadcast
            r16 = tmp.tile([1, ST], FP16, tag="r16", bufs=1, name="r16")
            nc.vector.tensor_copy(r16, r_row)
            prb = psum.tile([128, ST], F32, tag="pv", bufs=4, name="ps_rb")
            nc.tensor.matmul(prb, onesf_sb, r16, start=True, stop=True)
            nc.scalar.copy(rb_sb, prb)
        else:
            nc.gpsimd.partition_broadcast(rb_sb, r_row)
        nc.vector.tensor_mul(oT_sb[:, h, qsl], po, rb_sb)

    def emit_norm(h, qt, po, acc, use_pe=False):
        norm_stage2(norm_stage1(h, qt, po, acc), use_pe=use_pe)

    def outproj_gen(qt, wide=False):
        # wide=True (tail only): 4 PSUM tiles from the now-idle pv tag join
        # the rotation so copies never gate the matmul stream.
        def inner():
            for s128 in range(qt * 4, (qt + 1) * 4):
                if wide:
                    pouts = [
                        psum.tile([128, ST], F32, tag="pj", bufs=4, name="ps_out"),
                        psum.tile([128, ST], F32, tag="pj", bufs=4, name="ps_out"),
                        psum.tile([128, ST], F32, tag="pv", bufs=4, name="ps_out"),
                        psum.tile([128, ST], F32, tag="pv", bufs=4, name="ps_out"),
                    ]
                    for h in range(HPC):
                        for j in range(4):
                            nc.tensor.matmul(
                                pouts[j],
                                oT_sb[:, h, s128 * 128:(s128 + 1) * 128],
                                wo_sb[:, h, j * ST:(j + 1) * ST],
                                start=(h == 0), stop=(h == HPC - 1),
                            )
                        yield
                    for j in range(4):
                        o_sb = tmp.tile([128, ST], BF16, tag="osb", bufs=4, name="o_sb")
                        if j % 2 == 0:
                            nc.vector.tensor_copy(o_sb, pouts[j])
                        else:
                            nc.scalar.copy(o_sb, pouts[j])
                        eng = nc.sync if j % 2 == 0 else nc.scalar
                        eng.dma_start(
                            out=out[s128 * 128:(s128 + 1) * 128, j * ST:(j + 1) * ST],
                            in_=o_sb,
                        )
                        yield
                else:
                    for jp in range(2):
                        pouts = []
                        for jj in range(2):
                            pj_ = psum.tile(
                                [128, ST], F32, tag="pj", bufs=4, name="ps_out"
                            )
                            pouts.append(pj_)
                        for h in range(HPC):
                            for jj in range(2):
                                j = 2 * jp + jj
                                nc.tensor.matmul(
                                    pouts[jj],
                                    oT_sb[:, h, s128 * 128:(s128 + 1) * 128],
                                    wo_sb[:, h, j * ST:(j + 1) * ST],
                                    start=(h == 0), stop=(h == HPC - 1),
                                )
                            yield
                        for jj in range(2):
                            j = 2 * jp + jj
                            o_sb = tmp.tile([128, ST], BF16, tag="osb", bufs=4, name="o_sb")
                            if j % 2 == 0:
                                nc.vector.tensor_copy(o_sb, pouts[jj])
                            else:
                                nc.scalar.copy(o_sb, pouts[jj])
                            nc.sync.dma_start(
                                out=out[s128 * 128:(s128 + 1) * 128, j * ST:(j + 1) * ST],
                                in_=o_sb,
                            )
                        yield

        return inner()

    def drain(gen):
        for _ in gen:
            pass

    pend = [None]
    pull_acc = [0.0]

    def attn(qt, filler, pulls):
        for h in range(HPC):
            nkt = 4 * (qt + 1) if mode == "causal" else NKT
            po = psum.tile([128, ST], F32, tag="pj", bufs=4, name="ps_po")
            # (gpsimd acc-split was tried and reverted: gpsimd tensor ops
            # measure 1.1-2us each — the odd-kt chain lagged the ~900ns
            # tile cadence and the norm matmul stalled the in-order PE
            # queue 3-5us per head.)
            acc = tmp.tile([128, ST], FP16, tag="acc", bufs=2, name="acc")

            def emit_av(kt, q0, e_sb, po=po, h=h, nkt=nkt):
                nc.tensor.matmul(
                    po[:, q0:],
                    v_sb[:, kt, h * HD:(h + 1) * HD],
                    e_sb[:, q0:],
                    start=(kt == 0), stop=(kt == nkt - 1),
                )

            prev_av = None
            for kt in range(nkt):
                di = kt - 4 * qt
                # diagonal tiles only have valid queries at columns >= di*KT
                q0 = di * KT if (mode == "causal" and di >= 0) else 0
                qsl = slice(qt * ST + q0, (qt + 1) * ST)
                pss = psum.tile([128, ST], F32, tag="pv", bufs=4, name="ps_s")
                nc.tensor.matmul(
                    pss[:, q0:],
                    kT_sb[:, h, kt * KT:(kt + 1) * KT],
                    qT_sb[:, h, qsl],
                    start=True, stop=True,
                )
                e_sb = tmp.tile([128, ST], FP16, tag="e", bufs=7, name="e_sb")
                nc.scalar.activation(
                    e_sb[:, q0:], pss[:, q0:], func=EXP_FN,
                    scale=SCALE, bias=ebias_sb,
                )
                if mode == "causal":
                    if di >= 0:
                        nc.vector.tensor_mul(
                            e_sb[:, q0:], e_sb[:, q0:], mask_sb[:, di, q0:]
                        )
                elif mode == "masked":
                    m_sb = tmp.tile([128, ST], FP16, tag="m", bufs=4, name="m_sb")
                    nc.sync.dma_start(
                        out=m_sb, in_=emask[kt * KT:(kt + 1) * KT, qsl]
                    )
                    nc.vector.tensor_mul(e_sb, e_sb, m_sb)
                if kt == 0:
                    nc.vector.tensor_copy(acc, e_sb)
                else:
                    nc.vector.tensor_add(acc[:, q0:], acc[:, q0:], e_sb[:, q0:])
                if prev_av is not None:
                    emit_av(*prev_av)
                prev_av = (kt, q0, e_sb)
                if kt == 0 and pend[0] is not None:
                    # Flush the previous head's deferred norm AFTER this
                    # head's first score tile: the psm matmul then never
                    # heads the in-order PE queue while the previous
                    # head's DVE acc chain is still draining.
                    emit_norm(*pend[0])
                    pend[0] = None
                pull_acc[0] += pulls
                while pull_acc[0] >= 1.0:
                    pull_acc[0] -= 1.0
                    if next(filler, "end") == "end":
                        pull_acc[0] = 0.0
                        break
            emit_av(*prev_av)
            pend[0] = (h, qt, po, [acc])

    # pulls is yields-per-score-tile; sized so each generator lasts its
    # whole phase (dry filler = PE waits on the exp chain): yield counts
    # qproj=68, outproj=40 vs tiles 16/32/48/64.
    drain(qproj_gen(0))
    filler = qproj_gen(1)
    attn(0, filler, 4)
    drain(filler)
    filler = qproj_gen(2)
    attn(1, filler, 2)
    drain(filler)
    filler = outproj_gen(0)
    f2 = qproj_gen(3)
    import itertools
    filler = itertools.chain(f2, filler)
    attn(2, filler, 2)
    drain(filler)
    filler = itertools.chain(outproj_gen(1), outproj_gen(2))
    attn(3, filler, 1.25)
    drain(filler)
    emit_norm(*pend[0], use_pe=True)
    drain(outproj_gen(3, wide=True))


# ---------------------------------------------------------------------------
# Host side
# ---------------------------------------------------------------------------

_PROGRAMS: dict = {}


def _get_program(mode: str):
    if mode not in _PROGRAMS:
        _PROGRAMS[mode] = build_program(mode)
    return _PROGRAMS[mode]


_PERM = np.concatenate([np.arange(0, HD, 2), np.arange(1, HD, 2)])  # rotate-half


def _mask4_np() -> np.ndarray:
    m = np.zeros((4, KT, ST), dtype=np.float32)
    p = np.arange(KT)[:, None]
    qf = np.arange(ST)[None, :]
    for di in range(4):
        m[di] = (qf >= di * KT + p).astype(np.float32)
    return m.astype(NPFP16)


def _classify_mask(m: np.ndarray) -> str:
    if not np.any(m):
        return "dense"
    causal = np.triu(np.full((S, S), -1e9, dtype=np.float32), 1)
    if np.array_equal(m, causal):
        return "causal"
    return "masked"


def _pack_w(wT: np.ndarray, groups: int) -> np.ndarray:
    """[groups*128, M] -> [128, groups, M] partition-major contiguous."""
    g, m = groups, wT.shape[1]
    return np.ascontiguousarray(
        wT.reshape(g, 128, m).transpose(1, 0, 2)
    ).astype(NPBF16)


def make_in_maps(x, freqs_cos, freqs_sin, mask, wq, wk, wv, wo, mode):
    """Build the 8 per-core input dicts."""
    cosT = np.ascontiguousarray(np.asarray(freqs_cos, np.float32).T)  # [64, S]
    sinT = np.ascontiguousarray(np.asarray(freqs_sin, np.float32).T)
    cosT2 = np.concatenate([cosT, cosT], 0).astype(NPBF16)            # [128, S]
    # rows 0:64 get -sin (dst_e = qe*c - qo*s), rows 64:128 get +sin
    sinT2 = np.concatenate([-sinT, sinT], 0).astype(NPBF16)
    maskP = None
    if mode == "causal":
        maskP = np.ascontiguousarray(_mask4_np().transpose(1, 0, 2))  # [128,4,512]
    em = None
    if mode == "masked":
        # kernel indexes emask as [k, q]; mask is [q, k]
        em = np.exp(np.asarray(mask, np.float32).reshape(S, S)).T
        em = np.ascontiguousarray(em).astype(NPFP16)

    # permuted rows (within each head) for wq / wk
    perm_rows = (np.arange(H)[:, None] * HD + _PERM[None, :]).reshape(-1)
    wq_p = np.asarray(wq, np.float32)[perm_rows]
    wk_p = np.asarray(wk, np.float32)[perm_rows]
    wv_f = np.asarray(wv, np.float32)
    wo_f = np.asarray(wo, np.float32)
    x_f = np.asarray(x, np.float32)

    # x packed per batch: [128, NST, NC_CHUNK, ST]
    xPs = []
    for b in range(B):
        xPs.append(
            np.ascontiguousarray(
                x_f[b].reshape(NST, ST, NC_CHUNK, 128).transpose(3, 0, 2, 1)
            ).astype(NPBF16)
        )

    in_maps = []
    for core in range(NCORES):
        b, g = divmod(core, GROUPS)
        rs = slice(g * DPC, (g + 1) * DPC)
        im = {
            "xP": xPs[b],
            "wqP": _pack_w(np.ascontiguousarray(wq_p[rs].T), NC_CHUNK),
            "wkP": _pack_w(np.ascontiguousarray(wk_p[rs].T), NC_CHUNK),
            "wvP": _pack_w(np.ascontiguousarray(wv_f[rs].T), NC_CHUNK),
            "woP": _pack_w(np.ascontiguousarray(wo_f[:, rs].T), HPC),
            "cosT": cosT2,
            "sinT": sinT2,
        }
        if mode == "causal":
            im["maskP"] = maskP
        if mode == "masked":
            im["emask"] = em
        in_maps.append(im)
    return in_maps


def assemble(results) -> np.ndarray:
    out = np.zeros((B, S, D), dtype=np.float32)
    for core in range(NCORES):
        b = core // GROUPS
        out[b] += np.asarray(results[core]["out"], dtype=np.float32)
    return out


def kernel(x, freqs_cos, freqs_sin, mask, wq, wk, wv, wo, **run_kwargs):
    mode = _classify_mask(np.asarray(mask, np.float32).reshape(S, S))
    nc = _get_program(mode)
    in_maps = make_in_maps(x, freqs_cos, freqs_sin, mask, wq, wk, wv, wo, mode)
    res = run_bass_kernel_spmd(nc, in_maps, core_ids=list(range(NCORES)), **run_kwargs)
    out = assemble(res.results)
    kernel.last_results = res
    return out

